# revision 42
# baseline (speedup 1.0000x reference)
"""Trainium2 Bass kernel for causal multi-head attention (B=2, S=2048, D=1024, 16 heads x 64).

Sharding: 8 cores = 2 batches x 4 head-groups (tensor parallel over heads),
collective-free. Each core computes attention for its 4 heads over the full
sequence AND applies its own 4-head slice of W_O to all q positions,
producing a transposed partial output [D, S] that the host transposes and
sums across the 4 cores of each batch. Moving the head-sum "all-reduce" to
the host removes all 8 on-device AllGathers (7-26us each) plus the CC
barrier and their tail serialization that dominated the previous design.

Attention is flash-style with transposed scores: sT[k, q] = K Q^T (keys on
partitions). The two heads of a pair run as tile-positioned 64-row
stationaries on disjoint PE halves (concurrent), writing one [128, 1024]
2-bank PSUM tile so a single ACT exp covers both (two exps on deep-diagonal
rounds where the dead middle span would exceed an instruction overhead). AV
uses stationary [v | 1] so PSUM row 64 accumulates the softmax denominator
for free; the denominator row is broadcast to 64 partitions by a matmul and
reciprocated at base partition 0 with the fast custom DVE op. The
normalized outputs of a head pair are stacked on partitions 0-63 / 64-127
of a u2 tile (the odd head hops through a small SBUF->SBUF DMA since DVE
cannot cross partitions), which makes the out-projection contraction a
clean pair of full-128-deep matmuls per 128-col output block.

Scheduling: QKV projection and out-projection chains are chopped into
single-instruction units and drained between attention rounds as PE filler
(with a few units held in reserve across quarter boundaries). The last
quarter's out-projection is split into pass A (pair 0, runs as filler
during the final attention pass) and pass B (pair 1 + add, PSUM rotated
through 4 banks so DVE evacuation never stalls the matmul stream); half of
out_proj(2) is held back to cover the final norm + DMA-hop window. Inputs
stream only on the two HWDGE rings (sync/scalar; gpsimd dma_start is the
slow SWDGE path), chunk-interleaved so the first projection chain starts as
soon as chunk 0 lands. Dummy full-array matmuls during the initial DMA wait
and dry norm windows keep PE_HAM's activity monitor busy so the PE clock
gate stays at 8/8 (2.4 GHz) instead of the 4/8 (1.2 GHz) cold state —
HAM tracks array duty, so quadrant-sized warm-ups do not register.
"""

import os
import sys

sys.path.insert(0, "/opt/trn_rl_repo")

import numpy as np

# ---- problem constants (hardcoded; kernel.py must be self-contained) ----
B = 2
S = 2048
D = 1024
N_HEADS = 16
DH = 64                 # head dim
NCORES = 8
NH_CORE = N_HEADS // 4  # 4 heads per core (4-way TP x 2-way batch DP)
SCALE = 1.0 / 8.0       # 1/sqrt(64)

P = 128                 # partitions
DC = D // P             # 8 contraction chunks for the projections
KC = S // P             # 16 key chunks
QT = 512                # q tile width (free dim) per quarter
NQT = S // QT           # 4 q tiles
GRP = 4                 # cores per batch group

_CACHE = {}


def _build():
    import concourse.bass as bass
    import concourse.tile as tile
    from concourse import bacc, mybir

    f32 = mybir.dt.float32
    F16 = mybir.dt.float16

    nc = bacc.Bacc(
        "TRN2",
        target_bir_lowering=False,
        debug=False,
        enable_asserts=False,
        num_devices=NCORES,
    )

    # all inputs pre-chunked host-side to [128, DC, n] so DMAs are contiguous
    xt_d = nc.dram_tensor("xt", [P, DC, S], F16, kind="ExternalInput").ap()
    wqt_d = nc.dram_tensor("wqt", [P, DC, NH_CORE * DH], F16, kind="ExternalInput").ap()
    wkt_d = nc.dram_tensor("wkt", [P, DC, NH_CORE * DH], F16, kind="ExternalInput").ap()
    wvt_d = nc.dram_tensor("wvt", [P, DC, NH_CORE * DH], F16, kind="ExternalInput").ap()
    # own-head W_O, pair-stacked: [128 partitions = (even head hd | odd head
    # hd), pair, D]
    wop_d = nc.dram_tensor("wop", [P, 2, D], F16, kind="ExternalInput").ap()
    msk_d = nc.dram_tensor("msk", [P, P], F16, kind="ExternalInput").ap()
    # transposed partial output: [qt, dblock, 128 d, 512 q]
    out_d = nc.dram_tensor("out", [NQT * DC * P, QT], F16, kind="ExternalOutput").ap()

    Exp = mybir.ActivationFunctionType.Exp

    with tile.TileContext(nc) as tc:
        with (
            tc.tile_pool(name="const", bufs=1) as const,
            tc.tile_pool(name="work", bufs=2) as work,
            tc.tile_pool(name="ps", bufs=1, space="PSUM") as ps_pool,
        ):
            # ---------------- input DMAs ----------------
            # Only the sync/scalar HWDGE rings move real bandwidth (gpsimd
            # dma_start is the slow SWDGE Q7 path); the two rings stream in
            # parallel and share the ~358 GB/s HBM port. Order by first use:
            # the q/k chains gate on (wq|wk) + xt quarter 0, the v chains on
            # wv, everything else trails.
            # wq and xt quarter 0 land per d-chunk, interleaved across the two
            # rings, so the first projection chain starts as soon as chunk 0
            # arrives and is then paced by chunk arrivals (not whole tensors)
            wq_sb = const.tile([P, DC, NH_CORE * DH], F16)
            xt_sb = const.tile([P, DC, S], F16)
            for dc in range(0, DC, 2):
                nc.sync.dma_start(wq_sb[:, dc : dc + 2, :], wqt_d[:, dc : dc + 2, :])
                nc.scalar.dma_start(
                    xt_sb[:, dc : dc + 2, 0:QT], xt_d[:, dc : dc + 2, 0:QT]
                )
            wk_sb = const.tile([P, DC, NH_CORE * DH], F16)
            nc.sync.dma_start(wk_sb[:, 0:4, :], wkt_d[:, 0:4, :])
            nc.sync.dma_start(wk_sb[:, 4:DC, :], wkt_d[:, 4:DC, :])
            wv_sb = const.tile([P, DC, NH_CORE * DH], F16)
            nc.sync.dma_start(wv_sb[:], wvt_d)
            tri_sb = const.tile([P, P], F16)
            nc.sync.dma_start(tri_sb[:], msk_d)
            half = (S - QT) // 2
            nc.scalar.dma_start(
                xt_sb[:, :, QT : QT + half], xt_d[:, :, QT : QT + half]
            )
            nc.sync.dma_start(
                xt_sb[:, :, QT + half : S], xt_d[:, :, QT + half : S]
            )
            wo_sb = const.tile([P, 2, D], F16)
            nc.scalar.dma_start(wo_sb[:], wop_d)

            # ---------------- SBUF state ----------------
            qT = [const.tile([P, S], F16, name=f"qT{i}") for i in range(2)]
            kT = [const.tile([P, S], F16, name=f"kT{i}") for i in range(2)]
            v_aug = [const.tile([P, KC, DH + 1], F16, name=f"vaug{h}") for h in range(NH_CORE)]
            ones_f32 = const.tile([P, DH], f32)
            nc.vector.memset(ones_f32[:], 1.0)
            ones_f16 = const.tile([DH + 1, DH], F16)
            nc.vector.memset(ones_f16[:], 1.0)
            for h in range(NH_CORE):
                nc.gpsimd.tensor_copy(v_aug[h][:, :, DH : DH + 1], ones_f32[:, 0:KC, None])

            # PE_HAM warm-up: dependency-free dummy matmuls keep the PE's
            # activity monitor busy through DMA stalls and norm chains so the
            # clock gate stays at 8/8 (2.4 GHz) instead of dropping to 4/8
            warm_mov = const.tile([P, QT], F16)
            nc.vector.memset(warm_mov[:], 0.0)
            warm_stat = const.tile([P, P], F16)
            nc.vector.memset(warm_stat[:], 0.0)
            warm_ct = [0]

            def warm_mm():
                # full 128x128 stationary: HAM appears to track PE array duty,
                # so a quadrant-sized warm matmul doesn't register as busy.
                # sc-tagged: that buffer is only ever held briefly (scores ->
                # exp), so a warm matmul can never deadlock against a
                # long-held projection accumulator the way a pj tile could
                warm_ct[0] += 1
                wp = ps_pool.tile([P, QT], f32, name="warm", tag="sc", bufs=2)
                nc.tensor.matmul(
                    wp[:],
                    warm_stat[:],
                    warm_mov[:],
                    start=True,
                    stop=True,
                )

            for _ in range(3):
                warm_mm()

            # per-quarter normalized pair-stacked attention outputs
            u2 = {}

            pj = [0]  # alternating tag counter for the 2 shared psum banks

            def _pj_tile(shape, name):
                t = ps_pool.tile(shape, f32, name=name, tag=f"pj{pj[0] % 2}", bufs=1)
                pj[0] += 1
                return t

            # ---- filler units: single instructions emitted between rounds ----
            def qk_chain_units(nt, pr, w_sb, dst):
                st = {}
                us = []
                for dc in range(DC):
                    def mm(dc=dc, nt=nt, pr=pr, w_sb=w_sb):
                        if dc == 0:
                            st["pp"] = _pj_tile([P, QT], "pp")
                        nc.tensor.matmul(
                            st["pp"][:],
                            w_sb[:, dc, pr * P : (pr + 1) * P],
                            xt_sb[:, dc, nt * QT : (nt + 1) * QT],
                            start=(dc == 0),
                            stop=(dc == DC - 1),
                        )
                    us.append(mm)
                def cp(nt=nt, dst=dst):
                    nc.vector.tensor_copy(dst[:, nt * QT : (nt + 1) * QT], st["pp"][:])
                us.append(cp)
                return us

            def v_chain_units(pc):
                st = {}
                us = []
                for dc in range(DC):
                    def mm(dc=dc, pc=pc):
                        if dc == 0:
                            st["vp"] = _pj_tile([P, NH_CORE * DH], "vp")
                        nc.tensor.matmul(
                            st["vp"][:],
                            xt_sb[:, dc, pc * P : (pc + 1) * P],
                            wv_sb[:, dc, :],
                            start=(dc == 0),
                            stop=(dc == DC - 1),
                        )
                    us.append(mm)
                for h in range(NH_CORE):
                    def cp(h=h, pc=pc):
                        nc.vector.tensor_copy(
                            v_aug[h][:, pc, 0:DH], st["vp"][:, h * DH : (h + 1) * DH]
                        )
                    us.append(cp)
                return us

            def proj_units(nt, prs=(0, 1), with_v=True):
                us = []
                for pr in prs:
                    us += qk_chain_units(nt, pr, wq_sb, qT[pr])
                    us += qk_chain_units(nt, pr, wk_sb, kT[pr])
                if with_v:
                    for pc in range(4 * nt, 4 * nt + 4):
                        us += v_chain_units(pc)
                return us

            def out_proj_units(qt):
                """Own-head out-projection for quarter qt: per d-block, two
                full-128-deep accumulating matmuls (one per head pair) over
                the pair-stacked u2(qt), then evacuate + store transposed."""
                u2q = u2[qt]
                st = {}
                us = []
                for db in range(DC):
                    def mm0(db=db):
                        st["op"] = _pj_tile([P, QT], "op")
                        nc.tensor.matmul(
                            st["op"][:],
                            wo_sb[:, 0, db * P : (db + 1) * P],
                            u2q[:, 0, :],
                            start=True,
                            stop=False,
                        )
                    def mm1(db=db):
                        nc.tensor.matmul(
                            st["op"][:],
                            wo_sb[:, 1, db * P : (db + 1) * P],
                            u2q[:, 1, :],
                            start=False,
                            stop=True,
                        )
                    def cp(db=db, qt=qt):
                        if db % 2 == 0:
                            st["osb"] = work.tile([P, 2, QT], F16, name="osb2", bufs=4)
                        nc.vector.tensor_copy(st["osb"][:, db % 2, :], st["op"][:])
                    us += [mm0, mm1, cp]
                    if db % 2 == 1:
                        def dm(db=db, qt=qt):
                            row = (qt * DC + db - 1) * P
                            dst = out_d[row : row + 2 * P, :].rearrange(
                                "(a p) q -> p a q", p=P
                            )
                            nc.sync.dma_start(dst, st["osb"][:])
                        us.append(dm)
                return us

            def out_proj_a_units(qt):
                """Tail-shortening pass A for the last quarter: project pair 0
                (complete after pr=0's norm) into f16 partials while pr=1's
                attention still runs."""
                u2q = u2[qt]
                st = {}
                parts = []
                us = []
                for db in range(DC):
                    def mma(db=db):
                        st["op"] = _pj_tile([P, QT], "op")
                        nc.tensor.matmul(
                            st["op"][:],
                            wo_sb[:, 0, db * P : (db + 1) * P],
                            u2q[:, 0, :],
                            start=True,
                            stop=True,
                        )
                    def cpa(db=db):
                        part = work.tile([P, QT], F16, name="opart", bufs=DC)
                        parts.append(part)
                        nc.vector.tensor_copy(part[:], st["op"][:])
                    us += [mma, cpa]
                return us, parts

            def out_proj_b_units(qt, parts):
                """Pass B: pair 1 matmul + add pass-A partial, evacuate, store.
                The op tiles rotate through 4 PSUM banks (pj pair + the now
                idle acc banks) so the matmul stream never stalls on a DVE
                evacuation; stores go out per 2 d-blocks."""
                u2q = u2[qt]
                st = {}
                tags = ["pj0", "pj1", "acc0", "acc1"]
                us = []
                for db in range(DC):
                    def mmb(db=db):
                        st["op"] = ps_pool.tile(
                            [P, QT], f32, name="opb", tag=tags[db % 4], bufs=1
                        )
                        nc.tensor.matmul(
                            st["op"][:],
                            wo_sb[:, 1, db * P : (db + 1) * P],
                            u2q[:, 1, :],
                            start=True,
                            stop=True,
                        )
                    def cpb(db=db):
                        if db % 2 == 0:
                            st["osb"] = work.tile(
                                [P, 2, QT], F16, name="osb2", bufs=4
                            )
                        nc.vector.tensor_add(
                            st["osb"][:, db % 2, :], st["op"][:], parts[db][:]
                        )
                    us += [mmb, cpb]
                    if db % 2 == 1:
                        def dmb(db=db, qt=qt):
                            row = (qt * DC + db - 1) * P
                            dst = out_d[row : row + 2 * P, :].rearrange(
                                "(a p) q -> p a q", p=P
                            )
                            nc.sync.dma_start(dst, st["osb"][:])
                        us.append(dmb)
                return us

            units = []

            def fill(rounds_left):
                if not units:
                    return
                n = max(1, (len(units) + rounds_left - 1) // max(rounds_left, 1))
                for _ in range(min(n, len(units))):
                    units.pop(0)()

            def flush():
                while units:
                    units.pop(0)()

            def norm_store(qt, pr, accs, last=False):
                if not units:
                    # no filler left: pad the norm window with warm matmuls so
                    # PE_HAM doesn't see a low-duty window and re-throttle
                    for _ in range(2):
                        warm_mm()
                """Normalize the head pair and stack into u2[qt][:, pr, :]:
                even head straight from PSUM at lanes 0-63, odd head via a
                small SBUF->SBUF DMA hop to lanes 64-127 (scalar HWDGE ring;
                the gpsimd SWDGE queue's drain would gate the tail).

                For the last pair the whole chain is the kernel tail, so the
                odd head goes FIRST (its hop overlaps the even head's DVE
                mul) and one den evacuation moves to the idle ACT engine."""
                u2q = u2[qt]
                dens = {}
                for h2 in (1, 0) if last else (0, 1):
                    # f16 denominator stays on lane 64 (DVE can't move
                    # across partitions); the matmul streams from there
                    den = work.tile([DH + 1, QT], F16, name="den", bufs=6)
                    if last and h2 == 0:
                        nc.scalar.copy(den[DH : DH + 1, :], accs[h2][DH : DH + 1, :])
                    else:
                        nc.vector.tensor_copy(
                            den[DH : DH + 1, :], accs[h2][DH : DH + 1, :]
                        )
                    dens[h2] = den
                for h2 in (1, 0) if last else (0, 1):
                    rb_ps = _pj_tile([DH, QT], "rb")
                    nc.tensor.matmul(
                        rb_ps[:],
                        ones_f16[DH : DH + 1, :],
                        dens[h2][DH : DH + 1, :],
                        start=True,
                        stop=True,
                    )
                    rb_sb = work.tile([DH, QT], f32, name="rb_sb", bufs=4)
                    nc.vector.reciprocal_approx_fast(rb_sb[:], rb_ps[:])
                    if not units:
                        warm_mm()
                    if h2 == 0:
                        nc.vector.tensor_mul(
                            u2q[0:DH, pr, :], accs[h2][0:DH, :], rb_sb[:]
                        )
                    else:
                        tmp = work.tile([DH, QT], F16, name="u_tmp", bufs=2)
                        nc.vector.tensor_mul(tmp[:], accs[h2][0:DH, :], rb_sb[:])
                        nc.scalar.dma_start(u2q[DH:P, pr, :], tmp[:])

            # ---------------- main loop ----------------
            # upfront: only what gates qt0 pr0's first scores + first AV; the
            # remaining v chains and pr1's q/k chains become the first fillers
            for _ in range(7):
                warm_mm()
            for u in proj_units(0, prs=(0,), with_v=False) + v_chain_units(0):
                u()
            for qt in range(NQT):
                u2[qt] = work.tile([P, 2, QT], F16, name="u2", bufs=2)
                # filler work for this quarter's ACT-bound attention span
                if qt == 0:
                    for pc in range(1, 4):
                        units.extend(v_chain_units(pc))
                    units.extend(proj_units(0, prs=(1,), with_v=False))
                if qt + 1 < NQT:
                    units.extend(proj_units(qt + 1))
                if qt >= 1:
                    op_us = out_proj_units(qt - 1)
                    if qt == NQT - 1:
                        # hold back half of out_proj(2): it is the only
                        # dep-free PE work left to cover the last norm + DMA
                        # hop window after the final AV round
                        held_back = op_us[len(op_us) // 2 :]
                        op_us = op_us[: len(op_us) // 2]
                    units.extend(op_us)

                nk = (qt + 1) * (QT // P)
                # +3 slack keeps a few filler units in reserve past the last
                # round so the PE stays busy across the quarter-end norm
                # chain (a low-duty MID window would re-throttle HAM to 1.2
                # GHz for 3.4us)
                rounds_left = 2 * nk + 3
                for pr in range(2):
                    # full-partition shape so the tail out-projection can
                    # rotate its op tiles through these banks too
                    acc = [
                        ps_pool.tile(
                            [P, QT], f32, name=f"acc{h2}", tag=f"acc{h2}", bufs=1
                        )
                        for h2 in range(2)
                    ]

                    def av_round(kb, r, pat):
                        for h2 in range(2):
                            nc.tensor.matmul(
                                acc[h2][0 : DH + 1, r:QT],
                                v_aug[pr * 2 + h2][:, kb, :],
                                pat[:, h2 * QT + r : (h2 + 1) * QT],
                                start=(kb == 0),
                                stop=(kb == nk - 1),
                            )

                    pend = None  # (kb, r, pat) awaiting its AV matmuls
                    for kb in range(nk):
                        k_sl = slice(kb * P, (kb + 1) * P)
                        ri = kb - qt * (QT // P)  # >= 0 on diagonal tiles
                        r = max(ri, 0) * P        # first valid col in this q tile
                        c_sl = slice(qt * QT + r, (qt + 1) * QT)
                        sc = ps_pool.tile([P, 2 * QT], f32, name="sc", tag="sc", bufs=2)
                        pat = work.tile([P, 2 * QT], F16, name="pat", bufs=4)
                        for h2 in range(2):
                            hb = h2 * DH
                            # explicit row-group placement: the two 64-row
                            # stationaries occupy disjoint halves of the PE
                            # array so their LDWEIGHTS+MATMULs can overlap
                            nc.tensor.matmul(
                                sc[:, h2 * QT + r : (h2 + 1) * QT],
                                kT[pr][hb : hb + DH, k_sl],
                                qT[pr][hb : hb + DH, c_sl],
                                start=True,
                                stop=True,
                                tile_position=(hb, 0),
                            )
                        # one exp covers both heads when the dead span (cols
                        # 512..512+r hold stale-PSUM garbage, never read) is
                        # small; on deep-diagonal rounds two exps are cheaper
                        # than exp'ing r columns of garbage
                        if r >= 2 * P:
                            for h2 in range(2):
                                e_sl = slice(h2 * QT + r, (h2 + 1) * QT)
                                nc.scalar.activation(
                                    pat[:, e_sl], sc[:, e_sl], Exp, scale=SCALE
                                )
                        else:
                            nc.scalar.activation(
                                pat[:, r : 2 * QT], sc[:, r : 2 * QT], Exp, scale=SCALE
                            )
                        if ri >= 0:
                            for h2 in range(2):
                                nc.vector.tensor_mul(
                                    pat[:, h2 * QT + r : h2 * QT + r + P],
                                    pat[:, h2 * QT + r : h2 * QT + r + P],
                                    tri_sb[:],
                                )
                        # software-pipeline the PE stream one round deep
                        if pend is not None:
                            av_round(*pend)
                        pend = (kb, r, pat)
                        rounds_left -= 1
                        fill(rounds_left)
                    if pend is not None:
                        av_round(*pend)
                    norm_store(qt, pr, acc, last=(qt == NQT - 1 and pr == 1))
                    if qt == NQT - 1 and pr == 0:
                        a_us, op3_parts = out_proj_a_units(NQT - 1)
                        units.extend(a_us)
            for u in held_back:
                u()
            flush()
            for u in out_proj_b_units(NQT - 1, op3_parts):
                u()

    nc.compile()
    return nc


def _get_nc():
    if "nc" not in _CACHE:
        _CACHE["nc"] = _build()
    return _CACHE["nc"]


def _tri():
    k = np.arange(P)[:, None]
    q = np.arange(P)[None, :]
    return (q >= k).astype(np.float32)


def _ensure_ntff_hook():
    """Register the axon NTFF profile hook (missing antenv.axon_hooks shim)."""
    import types

    try:
        from antenv.axon_hooks import get_axon_ntff_profile_hook  # noqa: F401

        return
    except ImportError:
        pass
    import antenv

    if "/root/.axon_site" not in sys.path:
        sys.path.insert(0, "/root/.axon_site")
    from trn_agent_boot.trn_boot import _ntff_profile_via_ctypes

    hook = _ntff_profile_via_ctypes("/opt/axon/libaxon_pjrt.so")
    mod = types.ModuleType("antenv.axon_hooks")
    mod.get_axon_ntff_profile_hook = lambda: hook
    mod.set_axon_ntff_profile_hook = lambda h: None
    sys.modules["antenv.axon_hooks"] = mod
    antenv.axon_hooks = mod


def kernel(residual, W_Q, W_K, W_V, W_O):
    from concourse.bass_utils import run_bass_kernel_spmd

    if int(os.environ.get("KERNEL_TRACE", "0")):
        _ensure_ntff_hook()

    residual = np.ascontiguousarray(np.asarray(residual), np.float32)
    W_Q = np.ascontiguousarray(np.asarray(W_Q), np.float32)
    W_K = np.ascontiguousarray(np.asarray(W_K), np.float32)
    W_V = np.ascontiguousarray(np.asarray(W_V), np.float32)
    W_O = np.ascontiguousarray(np.asarray(W_O), np.float32)

    nc = _get_nc()
    tri = _tri()

    def chunked(a):
        # [D, n] -> [128, DC, n] so every DMA row is contiguous
        n = a.shape[1]
        return np.ascontiguousarray(
            a.reshape(DC, P, n).transpose(1, 0, 2).astype(np.float16)
        )

    in_maps = []
    for c in range(NCORES):
        b, g = divmod(c, GRP)
        hs = slice(g * NH_CORE, (g + 1) * NH_CORE)
        # pair-stack own-head W_O: [pair, odd, hd, D] -> [odd*64+hd, pair, D]
        wop = (
            W_O[hs]
            .reshape(2, 2, DH, D)
            .transpose(1, 2, 0, 3)
            .reshape(P, 2, D)
            .astype(np.float16)
        )
        in_maps.append(
            {
                "xt": chunked(residual[b].T),
                "wqt": chunked(W_Q[hs].transpose(2, 0, 1).reshape(D, NH_CORE * DH)),
                "wkt": chunked(W_K[hs].transpose(2, 0, 1).reshape(D, NH_CORE * DH)),
                "wvt": chunked(W_V[hs].transpose(2, 0, 1).reshape(D, NH_CORE * DH)),
                "wop": np.ascontiguousarray(wop),
                "msk": tri.astype(np.float16),
            }
        )

    res = run_bass_kernel_spmd(
        nc,
        in_maps,
        core_ids=list(range(NCORES)),
        trace=bool(int(os.environ.get("KERNEL_TRACE", "0"))),
        trace_cores=(
            list(range(NCORES))
            if int(os.environ.get("KERNEL_TRACE_ALL", "0"))
            else [0] if int(os.environ.get("KERNEL_TRACE", "0")) else None
        ),
    )
    _CACHE["last_results"] = res

    # host-side unshard: transpose each core's partial [qt, db, d, q] ->
    # [s, d] and sum the 4 head-group partials of each batch
    out = np.zeros((B, S, D), np.float32)
    for c in range(NCORES):
        b = c // GRP
        blk = np.asarray(res.results[c]["out"], np.float32).reshape(NQT, DC, P, QT)
        out[b] += blk.transpose(0, 3, 1, 2).reshape(S, D)
    return out


# revision 44
# speedup vs baseline: 1.0567x; 1.0567x over previous
"""Trainium2 Bass kernel for causal multi-head attention (B=2, S=2048, D=1024, 16 heads x 64).

Sharding: 8 cores = 2 batches x 4 head-groups (tensor parallel over heads),
collective-free. Each core computes attention for its 4 heads over the full
sequence AND applies its own 4-head slice of W_O to all q positions,
producing a transposed partial output [D, S] that the host transposes and
sums across the 4 cores of each batch. Moving the head-sum "all-reduce" to
the host removes all 8 on-device AllGathers (7-26us each) plus the CC
barrier and their tail serialization that dominated the previous design.

Attention is flash-style with transposed scores: sT[k, q] = K Q^T (keys on
partitions). The two heads of a pair run as tile-positioned 64-row
stationaries on disjoint PE halves (concurrent), writing one [128, 1024]
2-bank PSUM tile so a single ACT exp covers both (two exps on deep-diagonal
rounds where the dead middle span would exceed an instruction overhead). AV
uses stationary [v | 1] so PSUM row 64 accumulates the softmax denominator
for free; the denominator row is broadcast to 64 partitions by a matmul and
reciprocated at base partition 0 with the fast custom DVE op. The
normalized outputs of a head pair are stacked on partitions 0-63 / 64-127
of a u2 tile (the odd head hops through a small SBUF->SBUF DMA since DVE
cannot cross partitions), which makes the out-projection contraction a
clean pair of full-128-deep matmuls per 128-col output block.

Scheduling: QKV projection and out-projection chains are chopped into
single-instruction units and drained between attention rounds as PE filler
(with a few units held in reserve across quarter boundaries). The last
quarter's out-projection is split into pass A (pair 0, runs as filler
during the final attention pass) and pass B (pair 1 + add, PSUM rotated
through 4 banks so DVE evacuation never stalls the matmul stream); half of
out_proj(2) is held back to cover the final norm + DMA-hop window. Inputs
stream only on the two HWDGE rings (sync/scalar; gpsimd dma_start is the
slow SWDGE path), chunk-interleaved so the first projection chain starts as
soon as chunk 0 lands. Dummy full-array matmuls during the initial DMA wait
and dry norm windows keep PE_HAM's activity monitor busy so the PE clock
gate stays at 8/8 (2.4 GHz) instead of the 4/8 (1.2 GHz) cold state —
HAM tracks array duty, so quadrant-sized warm-ups do not register.
"""

import os
import sys

sys.path.insert(0, "/opt/trn_rl_repo")

import numpy as np

# ---- problem constants (hardcoded; kernel.py must be self-contained) ----
B = 2
S = 2048
D = 1024
N_HEADS = 16
DH = 64                 # head dim
NCORES = 8
NH_CORE = N_HEADS // 4  # 4 heads per core (4-way TP x 2-way batch DP)
SCALE = 1.0 / 8.0       # 1/sqrt(64)

P = 128                 # partitions
DC = D // P             # 8 contraction chunks for the projections
KC = S // P             # 16 key chunks
QT = 512                # q tile width (free dim) per quarter
NQT = S // QT           # 4 q tiles
GRP = 4                 # cores per batch group

_CACHE = {}


def _build():
    import concourse.bass as bass
    import concourse.tile as tile
    from concourse import bacc, mybir

    f32 = mybir.dt.float32
    F16 = mybir.dt.float16

    nc = bacc.Bacc(
        "TRN2",
        target_bir_lowering=False,
        debug=False,
        enable_asserts=False,
        num_devices=NCORES,
    )

    # all inputs pre-chunked host-side to [128, DC, n] so DMAs are contiguous
    xt_d = nc.dram_tensor("xt", [P, DC, S], F16, kind="ExternalInput").ap()
    wqt_d = nc.dram_tensor("wqt", [P, DC, NH_CORE * DH], F16, kind="ExternalInput").ap()
    wkt_d = nc.dram_tensor("wkt", [P, DC, NH_CORE * DH], F16, kind="ExternalInput").ap()
    wvt_d = nc.dram_tensor("wvt", [P, DC, NH_CORE * DH], F16, kind="ExternalInput").ap()
    # own-head W_O, pair-stacked: [128 partitions = (even head hd | odd head
    # hd), pair, D]
    wop_d = nc.dram_tensor("wop", [P, 2, D], F16, kind="ExternalInput").ap()
    msk_d = nc.dram_tensor("msk", [P, P], F16, kind="ExternalInput").ap()
    # transposed partial output: [qt, dblock, 128 d, 512 q]
    out_d = nc.dram_tensor("out", [NQT * DC * P, QT], F16, kind="ExternalOutput").ap()

    Exp = mybir.ActivationFunctionType.Exp

    with tile.TileContext(nc) as tc:
        with (
            tc.tile_pool(name="const", bufs=1) as const,
            tc.tile_pool(name="work", bufs=2) as work,
            tc.tile_pool(name="ps", bufs=1, space="PSUM") as ps_pool,
        ):
            # ---------------- input DMAs ----------------
            # Only the sync/scalar HWDGE rings move real bandwidth (gpsimd
            # dma_start is the slow SWDGE Q7 path); the two rings stream in
            # parallel and share the ~358 GB/s HBM port. Order by first use:
            # the q/k chains gate on (wq|wk) + xt quarter 0, the v chains on
            # wv, everything else trails.
            # wq and xt quarter 0 land per d-chunk, interleaved across the two
            # rings, so the first projection chain starts as soon as chunk 0
            # arrives and is then paced by chunk arrivals (not whole tensors)
            wq_sb = const.tile([P, DC, NH_CORE * DH], F16)
            xt_sb = const.tile([P, DC, S], F16)
            for dc in range(0, DC, 2):
                nc.sync.dma_start(wq_sb[:, dc : dc + 2, :], wqt_d[:, dc : dc + 2, :])
                nc.scalar.dma_start(
                    xt_sb[:, dc : dc + 2, 0:QT], xt_d[:, dc : dc + 2, 0:QT]
                )
            wk_sb = const.tile([P, DC, NH_CORE * DH], F16)
            nc.sync.dma_start(wk_sb[:, 0:4, :], wkt_d[:, 0:4, :])
            nc.sync.dma_start(wk_sb[:, 4:DC, :], wkt_d[:, 4:DC, :])
            wv_sb = const.tile([P, DC, NH_CORE * DH], F16)
            nc.sync.dma_start(wv_sb[:], wvt_d)
            tri_sb = const.tile([P, P], F16)
            nc.sync.dma_start(tri_sb[:], msk_d)
            half = (S - QT) // 2
            nc.scalar.dma_start(
                xt_sb[:, :, QT : QT + half], xt_d[:, :, QT : QT + half]
            )
            nc.sync.dma_start(
                xt_sb[:, :, QT + half : S], xt_d[:, :, QT + half : S]
            )
            wo_sb = const.tile([P, 2, D], F16)
            nc.scalar.dma_start(wo_sb[:], wop_d)

            # ---------------- SBUF state ----------------
            qT = [const.tile([P, S], F16, name=f"qT{i}") for i in range(2)]
            kT = [const.tile([P, S], F16, name=f"kT{i}") for i in range(2)]
            v_aug = [const.tile([P, KC, DH + 1], F16, name=f"vaug{h}") for h in range(NH_CORE)]
            ones_f32 = const.tile([P, DH], f32)
            nc.vector.memset(ones_f32[:], 1.0)
            ones_f16 = const.tile([DH + 1, DH], F16)
            nc.vector.memset(ones_f16[:], 1.0)
            for h in range(NH_CORE):
                nc.gpsimd.tensor_copy(v_aug[h][:, :, DH : DH + 1], ones_f32[:, 0:KC, None])

            # PE_HAM warm-up: dependency-free dummy matmuls keep the PE's
            # activity monitor busy through DMA stalls and norm chains so the
            # clock gate stays at 8/8 (2.4 GHz) instead of dropping to 4/8
            warm_mov = const.tile([P, QT], F16)
            nc.vector.memset(warm_mov[:], 0.0)
            warm_stat = const.tile([P, P], F16)
            nc.vector.memset(warm_stat[:], 0.0)
            warm_ct = [0]

            def warm_mm():
                # full 128x128 stationary: HAM appears to track PE array duty,
                # so a quadrant-sized warm matmul doesn't register as busy.
                # sc-tagged: that buffer is only ever held briefly (scores ->
                # exp), so a warm matmul can never deadlock against a
                # long-held projection accumulator the way a pj tile could
                warm_ct[0] += 1
                wp = ps_pool.tile([P, QT], f32, name="warm", tag="sc", bufs=2)
                nc.tensor.matmul(
                    wp[:],
                    warm_stat[:],
                    warm_mov[:],
                    start=True,
                    stop=True,
                )

            for _ in range(3):
                warm_mm()

            # per-quarter normalized pair-stacked attention outputs
            u2 = {}

            pj = [0]  # alternating tag counter for the 2 shared psum banks

            def _pj_tile(shape, name):
                t = ps_pool.tile(shape, f32, name=name, tag=f"pj{pj[0] % 2}", bufs=1)
                pj[0] += 1
                return t

            # ---- filler units: single instructions emitted between rounds ----
            def qk_chain_units(nt, pr, w_sb, dst):
                st = {}
                us = []
                for dc in range(DC):
                    def mm(dc=dc, nt=nt, pr=pr, w_sb=w_sb):
                        if dc == 0:
                            st["pp"] = _pj_tile([P, QT], "pp")
                        nc.tensor.matmul(
                            st["pp"][:],
                            w_sb[:, dc, pr * P : (pr + 1) * P],
                            xt_sb[:, dc, nt * QT : (nt + 1) * QT],
                            start=(dc == 0),
                            stop=(dc == DC - 1),
                        )
                    us.append(mm)
                def cp(nt=nt, dst=dst):
                    nc.vector.tensor_copy(dst[:, nt * QT : (nt + 1) * QT], st["pp"][:])
                us.append(cp)
                return us

            def v_chain_units(pc):
                st = {}
                us = []
                for dc in range(DC):
                    def mm(dc=dc, pc=pc):
                        if dc == 0:
                            st["vp"] = _pj_tile([P, NH_CORE * DH], "vp")
                        nc.tensor.matmul(
                            st["vp"][:],
                            xt_sb[:, dc, pc * P : (pc + 1) * P],
                            wv_sb[:, dc, :],
                            start=(dc == 0),
                            stop=(dc == DC - 1),
                        )
                    us.append(mm)
                for h in range(NH_CORE):
                    def cp(h=h, pc=pc):
                        nc.vector.tensor_copy(
                            v_aug[h][:, pc, 0:DH], st["vp"][:, h * DH : (h + 1) * DH]
                        )
                    us.append(cp)
                return us

            def proj_units(nt, prs=(0, 1), with_v=True):
                us = []
                for pr in prs:
                    us += qk_chain_units(nt, pr, wq_sb, qT[pr])
                    us += qk_chain_units(nt, pr, wk_sb, kT[pr])
                if with_v:
                    for pc in range(4 * nt, 4 * nt + 4):
                        us += v_chain_units(pc)
                return us

            def out_proj_units(qt):
                """Own-head out-projection for quarter qt: per d-block, two
                full-128-deep accumulating matmuls (one per head pair) over
                the pair-stacked u2(qt), then evacuate + store transposed."""
                u2q = u2[qt]
                st = {}
                us = []
                for db in range(DC):
                    def mm0(db=db):
                        st["op"] = _pj_tile([P, QT], "op")
                        nc.tensor.matmul(
                            st["op"][:],
                            wo_sb[:, 0, db * P : (db + 1) * P],
                            u2q[:, 0, :],
                            start=True,
                            stop=False,
                        )
                    def mm1(db=db):
                        nc.tensor.matmul(
                            st["op"][:],
                            wo_sb[:, 1, db * P : (db + 1) * P],
                            u2q[:, 1, :],
                            start=False,
                            stop=True,
                        )
                    def cp(db=db, qt=qt):
                        if db % 2 == 0:
                            st["osb"] = work.tile([P, 2, QT], F16, name="osb2", bufs=4)
                        nc.vector.tensor_copy(st["osb"][:, db % 2, :], st["op"][:])
                    us += [mm0, mm1, cp]
                    if db % 2 == 1:
                        def dm(db=db, qt=qt):
                            row = (qt * DC + db - 1) * P
                            dst = out_d[row : row + 2 * P, :].rearrange(
                                "(a p) q -> p a q", p=P
                            )
                            nc.sync.dma_start(dst, st["osb"][:])
                        us.append(dm)
                return us

            def out_proj_a_units(qt):
                """Tail-shortening pass A for the last quarter: project pair 0
                (complete after pr=0's norm) into f16 partials while pr=1's
                attention still runs."""
                u2q = u2[qt]
                st = {}
                parts = []
                us = []
                for db in range(DC):
                    def mma(db=db):
                        st["op"] = _pj_tile([P, QT], "op")
                        nc.tensor.matmul(
                            st["op"][:],
                            wo_sb[:, 0, db * P : (db + 1) * P],
                            u2q[:, 0, :],
                            start=True,
                            stop=True,
                        )
                    def cpa(db=db):
                        part = work.tile([P, QT], F16, name="opart", bufs=DC)
                        parts.append(part)
                        nc.vector.tensor_copy(part[:], st["op"][:])
                    us += [mma, cpa]
                return us, parts

            def out_proj_b_units(qt, parts):
                """Pass B: pair 1 matmul + add pass-A partial, evacuate, store.
                The op tiles rotate through 4 PSUM banks (pj pair + the now
                idle acc banks) so the matmul stream never stalls on a DVE
                evacuation; stores go out per 2 d-blocks."""
                u2q = u2[qt]
                st = {}
                tags = ["pj0", "pj1", "acc0", "acc1"]
                us = []
                for db in range(DC):
                    def mmb(db=db):
                        st["op"] = ps_pool.tile(
                            [P, QT], f32, name="opb", tag=tags[db % 4], bufs=1
                        )
                        nc.tensor.matmul(
                            st["op"][:],
                            wo_sb[:, 1, db * P : (db + 1) * P],
                            u2q[:, 1, :],
                            start=True,
                            stop=True,
                        )
                    def cpb(db=db):
                        if db % 2 == 0:
                            st["osb"] = work.tile(
                                [P, 2, QT], F16, name="osb2", bufs=4
                            )
                        nc.vector.tensor_add(
                            st["osb"][:, db % 2, :], st["op"][:], parts[db][:]
                        )
                    us += [mmb, cpb]
                    if db % 2 == 1:
                        def dmb(db=db, qt=qt):
                            row = (qt * DC + db - 1) * P
                            dst = out_d[row : row + 2 * P, :].rearrange(
                                "(a p) q -> p a q", p=P
                            )
                            nc.sync.dma_start(dst, st["osb"][:])
                        us.append(dmb)
                return us

            units = []

            def fill(rounds_left):
                if not units:
                    return
                n = max(1, (len(units) + rounds_left - 1) // max(rounds_left, 1))
                for _ in range(min(n, len(units))):
                    units.pop(0)()

            def flush():
                while units:
                    units.pop(0)()

            def norm_store(qt, pr, accs, last=False):
                if not units:
                    # no filler left: pad the norm window with warm matmuls so
                    # PE_HAM doesn't see a low-duty window and re-throttle
                    for _ in range(2):
                        warm_mm()
                """Normalize the head pair and stack into u2[qt][:, pr, :]:
                even head straight from PSUM at lanes 0-63, odd head via a
                small SBUF->SBUF DMA hop to lanes 64-127 (scalar HWDGE ring;
                the gpsimd SWDGE queue's drain would gate the tail).

                For the last pair the whole chain is the kernel tail, so the
                odd head goes FIRST (its hop overlaps the even head's DVE
                mul) and one den evacuation moves to the idle ACT engine."""
                u2q = u2[qt]
                dens = {}
                for h2 in (1, 0) if last else (0, 1):
                    # f16 denominator stays on lane 64 (DVE can't move
                    # across partitions); the matmul streams from there
                    den = work.tile([DH + 1, QT], F16, name="den", bufs=6)
                    if last and h2 == 0:
                        nc.scalar.copy(den[DH : DH + 1, :], accs[h2][DH : DH + 1, :])
                    else:
                        nc.vector.tensor_copy(
                            den[DH : DH + 1, :], accs[h2][DH : DH + 1, :]
                        )
                    dens[h2] = den
                for h2 in (1, 0) if last else (0, 1):
                    rb_ps = _pj_tile([DH, QT], "rb")
                    nc.tensor.matmul(
                        rb_ps[:],
                        ones_f16[DH : DH + 1, :],
                        dens[h2][DH : DH + 1, :],
                        start=True,
                        stop=True,
                    )
                    rb_sb = work.tile([DH, QT], f32, name="rb_sb", bufs=4)
                    nc.vector.reciprocal_approx_fast(rb_sb[:], rb_ps[:])
                    if not units:
                        warm_mm()
                    if h2 == 0:
                        nc.vector.tensor_mul(
                            u2q[0:DH, pr, :], accs[h2][0:DH, :], rb_sb[:]
                        )
                    else:
                        tmp = work.tile([DH, QT], F16, name="u_tmp", bufs=2)
                        nc.vector.tensor_mul(tmp[:], accs[h2][0:DH, :], rb_sb[:])
                        nc.scalar.dma_start(u2q[DH:P, pr, :], tmp[:])

            # ---------------- main loop ----------------
            # upfront: only what gates qt0 pr0's first scores + first AV; the
            # remaining v chains and pr1's q/k chains become the first fillers
            for _ in range(7):
                warm_mm()
            for u in proj_units(0, prs=(0,), with_v=False) + v_chain_units(0):
                u()
            for qt in range(NQT):
                u2[qt] = work.tile([P, 2, QT], F16, name="u2", bufs=2)
                # filler work for this quarter's ACT-bound attention span
                if qt == 0:
                    for pc in range(1, 4):
                        units.extend(v_chain_units(pc))
                    units.extend(proj_units(0, prs=(1,), with_v=False))
                if qt + 1 < NQT:
                    units.extend(proj_units(qt + 1))
                if qt >= 1:
                    op_us = out_proj_units(qt - 1)
                    if qt == NQT - 1:
                        # hold back half of out_proj(2): it is the only
                        # dep-free PE work left to cover the last norm + DMA
                        # hop window after the final AV round
                        held_back = op_us[len(op_us) // 2 :]
                        op_us = op_us[: len(op_us) // 2]
                    units.extend(op_us)

                nk = (qt + 1) * (QT // P)
                # +3 slack keeps a few filler units in reserve past the last
                # round so the PE stays busy across the quarter-end norm
                # chain (a low-duty MID window would re-throttle HAM to 1.2
                # GHz for 3.4us)
                rounds_left = 2 * nk + 3
                for pr in range(2):
                    # full-partition shape so the tail out-projection can
                    # rotate its op tiles through these banks too
                    acc = [
                        ps_pool.tile(
                            [P, QT], f32, name=f"acc{h2}", tag=f"acc{h2}", bufs=1
                        )
                        for h2 in range(2)
                    ]

                    def av_round(kb, r, pat):
                        for h2 in range(2):
                            nc.tensor.matmul(
                                acc[h2][0 : DH + 1, r:QT],
                                v_aug[pr * 2 + h2][:, kb, :],
                                pat[:, h2 * QT + r : (h2 + 1) * QT],
                                start=(kb == 0),
                                stop=(kb == nk - 1),
                            )

                    pend = []  # [(kb, r, pat), ...] awaiting AV matmuls
                    for kb in range(nk):
                        k_sl = slice(kb * P, (kb + 1) * P)
                        ri = kb - qt * (QT // P)  # >= 0 on diagonal tiles
                        r = max(ri, 0) * P        # first valid col in this q tile
                        c_sl = slice(qt * QT + r, (qt + 1) * QT)
                        sc = ps_pool.tile([P, 2 * QT], f32, name="sc", tag="sc", bufs=2)
                        pat = work.tile([P, 2 * QT], F16, name="pat", bufs=4)
                        for h2 in range(2):
                            hb = h2 * DH
                            # explicit row-group placement: the two 64-row
                            # stationaries occupy disjoint halves of the PE
                            # array so their LDWEIGHTS+MATMULs can overlap
                            nc.tensor.matmul(
                                sc[:, h2 * QT + r : (h2 + 1) * QT],
                                kT[pr][hb : hb + DH, k_sl],
                                qT[pr][hb : hb + DH, c_sl],
                                start=True,
                                stop=True,
                                tile_position=(hb, 0),
                            )
                        # one exp covers both heads when the dead span (cols
                        # 512..512+r hold stale-PSUM garbage, never read) is
                        # small; on deep-diagonal rounds two exps are cheaper
                        # than exp'ing r columns of garbage
                        if r >= 2 * P:
                            for h2 in range(2):
                                e_sl = slice(h2 * QT + r, (h2 + 1) * QT)
                                nc.scalar.activation(
                                    pat[:, e_sl], sc[:, e_sl], Exp, scale=SCALE
                                )
                        else:
                            nc.scalar.activation(
                                pat[:, r : 2 * QT], sc[:, r : 2 * QT], Exp, scale=SCALE
                            )
                        if ri >= 0:
                            for h2 in range(2):
                                nc.vector.tensor_mul(
                                    pat[:, h2 * QT + r : h2 * QT + r + P],
                                    pat[:, h2 * QT + r : h2 * QT + r + P],
                                    tri_sb[:],
                                )
                        # software-pipeline the PE stream two rounds deep so
                        # the exp's ACT latency never exposes on the PE
                        pend.append((kb, r, pat))
                        if len(pend) > 2:
                            av_round(*pend.pop(0))
                        rounds_left -= 1
                        fill(rounds_left)
                    while pend:
                        av_round(*pend.pop(0))
                    norm_store(qt, pr, acc, last=(qt == NQT - 1 and pr == 1))
                    if qt == NQT - 1 and pr == 0:
                        a_us, op3_parts = out_proj_a_units(NQT - 1)
                        units.extend(a_us)
            for u in held_back:
                u()
            flush()
            for u in out_proj_b_units(NQT - 1, op3_parts):
                u()

    nc.compile()
    return nc


def _get_nc():
    if "nc" not in _CACHE:
        _CACHE["nc"] = _build()
    return _CACHE["nc"]


def _tri():
    k = np.arange(P)[:, None]
    q = np.arange(P)[None, :]
    return (q >= k).astype(np.float32)


def _ensure_ntff_hook():
    """Register the axon NTFF profile hook (missing antenv.axon_hooks shim)."""
    import types

    try:
        from antenv.axon_hooks import get_axon_ntff_profile_hook  # noqa: F401

        return
    except ImportError:
        pass
    import antenv

    if "/root/.axon_site" not in sys.path:
        sys.path.insert(0, "/root/.axon_site")
    from trn_agent_boot.trn_boot import _ntff_profile_via_ctypes

    hook = _ntff_profile_via_ctypes("/opt/axon/libaxon_pjrt.so")
    mod = types.ModuleType("antenv.axon_hooks")
    mod.get_axon_ntff_profile_hook = lambda: hook
    mod.set_axon_ntff_profile_hook = lambda h: None
    sys.modules["antenv.axon_hooks"] = mod
    antenv.axon_hooks = mod


def kernel(residual, W_Q, W_K, W_V, W_O):
    from concourse.bass_utils import run_bass_kernel_spmd

    if int(os.environ.get("KERNEL_TRACE", "0")):
        _ensure_ntff_hook()

    residual = np.ascontiguousarray(np.asarray(residual), np.float32)
    W_Q = np.ascontiguousarray(np.asarray(W_Q), np.float32)
    W_K = np.ascontiguousarray(np.asarray(W_K), np.float32)
    W_V = np.ascontiguousarray(np.asarray(W_V), np.float32)
    W_O = np.ascontiguousarray(np.asarray(W_O), np.float32)

    nc = _get_nc()
    tri = _tri()

    def chunked(a):
        # [D, n] -> [128, DC, n] so every DMA row is contiguous
        n = a.shape[1]
        return np.ascontiguousarray(
            a.reshape(DC, P, n).transpose(1, 0, 2).astype(np.float16)
        )

    in_maps = []
    for c in range(NCORES):
        b, g = divmod(c, GRP)
        hs = slice(g * NH_CORE, (g + 1) * NH_CORE)
        # pair-stack own-head W_O: [pair, odd, hd, D] -> [odd*64+hd, pair, D]
        wop = (
            W_O[hs]
            .reshape(2, 2, DH, D)
            .transpose(1, 2, 0, 3)
            .reshape(P, 2, D)
            .astype(np.float16)
        )
        in_maps.append(
            {
                "xt": chunked(residual[b].T),
                "wqt": chunked(W_Q[hs].transpose(2, 0, 1).reshape(D, NH_CORE * DH)),
                "wkt": chunked(W_K[hs].transpose(2, 0, 1).reshape(D, NH_CORE * DH)),
                "wvt": chunked(W_V[hs].transpose(2, 0, 1).reshape(D, NH_CORE * DH)),
                "wop": np.ascontiguousarray(wop),
                "msk": tri.astype(np.float16),
            }
        )

    res = run_bass_kernel_spmd(
        nc,
        in_maps,
        core_ids=list(range(NCORES)),
        trace=bool(int(os.environ.get("KERNEL_TRACE", "0"))),
        trace_cores=(
            list(range(NCORES))
            if int(os.environ.get("KERNEL_TRACE_ALL", "0"))
            else [0] if int(os.environ.get("KERNEL_TRACE", "0")) else None
        ),
    )
    _CACHE["last_results"] = res

    # host-side unshard: transpose each core's partial [qt, db, d, q] ->
    # [s, d] and sum the 4 head-group partials of each batch
    out = np.zeros((B, S, D), np.float32)
    for c in range(NCORES):
        b = c // GRP
        blk = np.asarray(res.results[c]["out"], np.float32).reshape(NQT, DC, P, QT)
        out[b] += blk.transpose(0, 3, 1, 2).reshape(S, D)
    return out


# revision 45
# speedup vs baseline: 1.0651x; 1.0079x over previous
"""Trainium2 Bass kernel for causal multi-head attention (B=2, S=2048, D=1024, 16 heads x 64).

Sharding: 8 cores = 2 batches x 4 head-groups (tensor parallel over heads),
collective-free. Each core computes attention for its 4 heads over the full
sequence AND applies its own 4-head slice of W_O to all q positions,
producing a transposed partial output [D, S] that the host transposes and
sums across the 4 cores of each batch. Moving the head-sum "all-reduce" to
the host removes all 8 on-device AllGathers (7-26us each) plus the CC
barrier and their tail serialization that dominated the previous design.

Attention is flash-style with transposed scores: sT[k, q] = K Q^T (keys on
partitions). The two heads of a pair run as tile-positioned 64-row
stationaries on disjoint PE halves (concurrent), writing one [128, 1024]
2-bank PSUM tile so a single ACT exp covers both (two exps on deep-diagonal
rounds where the dead middle span would exceed an instruction overhead). AV
uses stationary [v | 1] so PSUM row 64 accumulates the softmax denominator
for free; the denominator row is broadcast to 64 partitions by a matmul and
reciprocated at base partition 0 with the fast custom DVE op. The
normalized outputs of a head pair are stacked on partitions 0-63 / 64-127
of a u2 tile (the odd head hops through a small SBUF->SBUF DMA since DVE
cannot cross partitions), which makes the out-projection contraction a
clean pair of full-128-deep matmuls per 128-col output block.

Scheduling: QKV projection and out-projection chains are chopped into
single-instruction units and drained between attention rounds as PE filler
(with a few units held in reserve across quarter boundaries). The last
quarter's out-projection is split into pass A (pair 0, runs as filler
during the final attention pass) and pass B (pair 1 + add, PSUM rotated
through 4 banks so DVE evacuation never stalls the matmul stream); half of
out_proj(2) is held back to cover the final norm + DMA-hop window. Inputs
stream only on the two HWDGE rings (sync/scalar; gpsimd dma_start is the
slow SWDGE path), chunk-interleaved so the first projection chain starts as
soon as chunk 0 lands. Dummy full-array matmuls during the initial DMA wait
and dry norm windows keep PE_HAM's activity monitor busy so the PE clock
gate stays at 8/8 (2.4 GHz) instead of the 4/8 (1.2 GHz) cold state —
HAM tracks array duty, so quadrant-sized warm-ups do not register.
"""

import os
import sys

sys.path.insert(0, "/opt/trn_rl_repo")

import numpy as np

# ---- problem constants (hardcoded; kernel.py must be self-contained) ----
B = 2
S = 2048
D = 1024
N_HEADS = 16
DH = 64                 # head dim
NCORES = 8
NH_CORE = N_HEADS // 4  # 4 heads per core (4-way TP x 2-way batch DP)
SCALE = 1.0 / 8.0       # 1/sqrt(64)

P = 128                 # partitions
DC = D // P             # 8 contraction chunks for the projections
KC = S // P             # 16 key chunks
QT = 512                # q tile width (free dim) per quarter
NQT = S // QT           # 4 q tiles
GRP = 4                 # cores per batch group

_CACHE = {}


def _build():
    import concourse.bass as bass
    import concourse.tile as tile
    from concourse import bacc, mybir

    f32 = mybir.dt.float32
    F16 = mybir.dt.float16

    nc = bacc.Bacc(
        "TRN2",
        target_bir_lowering=False,
        debug=False,
        enable_asserts=False,
        num_devices=NCORES,
    )

    # all inputs pre-chunked host-side to [128, DC, n] so DMAs are contiguous
    xt_d = nc.dram_tensor("xt", [P, DC, S], F16, kind="ExternalInput").ap()
    wqt_d = nc.dram_tensor("wqt", [P, DC, NH_CORE * DH], F16, kind="ExternalInput").ap()
    wkt_d = nc.dram_tensor("wkt", [P, DC, NH_CORE * DH], F16, kind="ExternalInput").ap()
    wvt_d = nc.dram_tensor("wvt", [P, DC, NH_CORE * DH], F16, kind="ExternalInput").ap()
    # own-head W_O, pair-stacked: [128 partitions = (even head hd | odd head
    # hd), pair, D]
    wop_d = nc.dram_tensor("wop", [P, 2, D], F16, kind="ExternalInput").ap()
    msk_d = nc.dram_tensor("msk", [P, P], F16, kind="ExternalInput").ap()
    # transposed partial output: [qt, dblock, 128 d, 512 q]
    out_d = nc.dram_tensor("out", [NQT * DC * P, QT], F16, kind="ExternalOutput").ap()

    Exp = mybir.ActivationFunctionType.Exp

    with tile.TileContext(nc) as tc:
        with (
            tc.tile_pool(name="const", bufs=1) as const,
            tc.tile_pool(name="work", bufs=2) as work,
            tc.tile_pool(name="ps", bufs=1, space="PSUM") as ps_pool,
        ):
            # ---------------- input DMAs ----------------
            # Only the sync/scalar HWDGE rings move real bandwidth (gpsimd
            # dma_start is the slow SWDGE Q7 path); the two rings stream in
            # parallel and share the ~358 GB/s HBM port. Order by first use:
            # the q/k chains gate on (wq|wk) + xt quarter 0, the v chains on
            # wv, everything else trails.
            # wq and xt quarter 0 land per d-chunk, interleaved across the two
            # rings, so the first projection chain starts as soon as chunk 0
            # arrives and is then paced by chunk arrivals (not whole tensors)
            wq_sb = const.tile([P, DC, NH_CORE * DH], F16)
            xt_sb = const.tile([P, DC, S], F16)
            for dc in range(0, DC, 2):
                nc.sync.dma_start(wq_sb[:, dc : dc + 2, :], wqt_d[:, dc : dc + 2, :])
                nc.scalar.dma_start(
                    xt_sb[:, dc : dc + 2, 0:QT], xt_d[:, dc : dc + 2, 0:QT]
                )
            wk_sb = const.tile([P, DC, NH_CORE * DH], F16)
            nc.sync.dma_start(wk_sb[:, 0:4, :], wkt_d[:, 0:4, :])
            nc.sync.dma_start(wk_sb[:, 4:DC, :], wkt_d[:, 4:DC, :])
            wv_sb = const.tile([P, DC, NH_CORE * DH], F16)
            nc.sync.dma_start(wv_sb[:], wvt_d)
            tri_sb = const.tile([P, P], F16)
            nc.sync.dma_start(tri_sb[:], msk_d)
            half = (S - QT) // 2
            nc.scalar.dma_start(
                xt_sb[:, :, QT : QT + half], xt_d[:, :, QT : QT + half]
            )
            nc.sync.dma_start(
                xt_sb[:, :, QT + half : S], xt_d[:, :, QT + half : S]
            )
            wo_sb = const.tile([P, 2, D], F16)
            nc.scalar.dma_start(wo_sb[:], wop_d)

            # ---------------- SBUF state ----------------
            qT = [const.tile([P, S], F16, name=f"qT{i}") for i in range(2)]
            kT = [const.tile([P, S], F16, name=f"kT{i}") for i in range(2)]
            v_aug = [const.tile([P, KC, DH + 1], F16, name=f"vaug{h}") for h in range(NH_CORE)]
            ones_f32 = const.tile([P, DH], f32)
            nc.vector.memset(ones_f32[:], 1.0)
            ones_f16 = const.tile([DH + 1, DH], F16)
            nc.vector.memset(ones_f16[:], 1.0)
            for h in range(NH_CORE):
                nc.gpsimd.tensor_copy(v_aug[h][:, :, DH : DH + 1], ones_f32[:, 0:KC, None])

            # PE_HAM warm-up: dependency-free dummy matmuls keep the PE's
            # activity monitor busy through DMA stalls and norm chains so the
            # clock gate stays at 8/8 (2.4 GHz) instead of dropping to 4/8
            warm_mov = const.tile([P, QT], F16)
            nc.vector.memset(warm_mov[:], 0.0)
            warm_stat = const.tile([P, P], F16)
            nc.vector.memset(warm_stat[:], 0.0)
            warm_ct = [0]

            def warm_mm():
                # full 128x128 stationary: HAM appears to track PE array duty,
                # so a quadrant-sized warm matmul doesn't register as busy.
                # sc-tagged: that buffer is only ever held briefly (scores ->
                # exp), so a warm matmul can never deadlock against a
                # long-held projection accumulator the way a pj tile could
                warm_ct[0] += 1
                wp = ps_pool.tile([P, QT], f32, name="warm", tag="sc", bufs=2)
                nc.tensor.matmul(
                    wp[:],
                    warm_stat[:],
                    warm_mov[:],
                    start=True,
                    stop=True,
                )

            for _ in range(3):
                warm_mm()

            # per-quarter normalized pair-stacked attention outputs
            u2 = {}

            pj = [0]  # alternating tag counter for the 2 shared psum banks

            def _pj_tile(shape, name):
                t = ps_pool.tile(shape, f32, name=name, tag=f"pj{pj[0] % 2}", bufs=1)
                pj[0] += 1
                return t

            # ---- filler units: single instructions emitted between rounds ----
            def qk_chain_units(nt, pr, w_sb, dst):
                st = {}
                us = []
                for dc in range(DC):
                    def mm(dc=dc, nt=nt, pr=pr, w_sb=w_sb):
                        if dc == 0:
                            st["pp"] = _pj_tile([P, QT], "pp")
                        nc.tensor.matmul(
                            st["pp"][:],
                            w_sb[:, dc, pr * P : (pr + 1) * P],
                            xt_sb[:, dc, nt * QT : (nt + 1) * QT],
                            start=(dc == 0),
                            stop=(dc == DC - 1),
                        )
                    us.append(mm)
                def cp(nt=nt, dst=dst):
                    nc.vector.tensor_copy(dst[:, nt * QT : (nt + 1) * QT], st["pp"][:])
                us.append(cp)
                return us

            def v_chain_units(pc):
                st = {}
                us = []
                for dc in range(DC):
                    def mm(dc=dc, pc=pc):
                        if dc == 0:
                            st["vp"] = _pj_tile([P, NH_CORE * DH], "vp")
                        nc.tensor.matmul(
                            st["vp"][:],
                            xt_sb[:, dc, pc * P : (pc + 1) * P],
                            wv_sb[:, dc, :],
                            start=(dc == 0),
                            stop=(dc == DC - 1),
                        )
                    us.append(mm)
                for h in range(NH_CORE):
                    def cp(h=h, pc=pc):
                        nc.vector.tensor_copy(
                            v_aug[h][:, pc, 0:DH], st["vp"][:, h * DH : (h + 1) * DH]
                        )
                    us.append(cp)
                return us

            def proj_units(nt, prs=(0, 1), with_v=True):
                us = []
                for pr in prs:
                    us += qk_chain_units(nt, pr, wq_sb, qT[pr])
                    us += qk_chain_units(nt, pr, wk_sb, kT[pr])
                if with_v:
                    for pc in range(4 * nt, 4 * nt + 4):
                        us += v_chain_units(pc)
                return us

            def out_proj_units(qt):
                """Own-head out-projection for quarter qt: per d-block, two
                full-128-deep accumulating matmuls (one per head pair) over
                the pair-stacked u2(qt), then evacuate + store transposed."""
                u2q = u2[qt]
                st = {}
                us = []
                for db in range(DC):
                    def mm0(db=db):
                        st["op"] = _pj_tile([P, QT], "op")
                        nc.tensor.matmul(
                            st["op"][:],
                            wo_sb[:, 0, db * P : (db + 1) * P],
                            u2q[:, 0, :],
                            start=True,
                            stop=False,
                        )
                    def mm1(db=db):
                        nc.tensor.matmul(
                            st["op"][:],
                            wo_sb[:, 1, db * P : (db + 1) * P],
                            u2q[:, 1, :],
                            start=False,
                            stop=True,
                        )
                    def cp(db=db, qt=qt):
                        if db % 2 == 0:
                            st["osb"] = work.tile([P, 2, QT], F16, name="osb2", bufs=4)
                        nc.vector.tensor_copy(st["osb"][:, db % 2, :], st["op"][:])
                    us += [mm0, mm1, cp]
                    if db % 2 == 1:
                        def dm(db=db, qt=qt):
                            row = (qt * DC + db - 1) * P
                            dst = out_d[row : row + 2 * P, :].rearrange(
                                "(a p) q -> p a q", p=P
                            )
                            nc.sync.dma_start(dst, st["osb"][:])
                        us.append(dm)
                return us

            def out_proj_a_units(qt):
                """Tail-shortening pass A for the last quarter: project pair 0
                (complete after pr=0's norm) into f16 partials while pr=1's
                attention still runs."""
                u2q = u2[qt]
                st = {}
                parts = []
                us = []
                for db in range(DC):
                    def mma(db=db):
                        st["op"] = _pj_tile([P, QT], "op")
                        nc.tensor.matmul(
                            st["op"][:],
                            wo_sb[:, 0, db * P : (db + 1) * P],
                            u2q[:, 0, :],
                            start=True,
                            stop=True,
                        )
                    def cpa(db=db):
                        part = work.tile([P, QT], F16, name="opart", bufs=DC)
                        parts.append(part)
                        nc.vector.tensor_copy(part[:], st["op"][:])
                    us += [mma, cpa]
                return us, parts

            def out_proj_b_units(qt, parts):
                """Pass B: pair 1 matmul + add pass-A partial, evacuate, store.
                The op tiles rotate through 4 PSUM banks (pj pair + the now
                idle acc banks) so the matmul stream never stalls on a DVE
                evacuation; stores go out per 2 d-blocks."""
                u2q = u2[qt]
                st = {}
                tags = ["pj0", "pj1", "acc0", "acc1"]
                us = []
                for db in range(DC):
                    def mmb(db=db):
                        st["op"] = ps_pool.tile(
                            [P, QT], f32, name="opb", tag=tags[db % 4], bufs=1
                        )
                        nc.tensor.matmul(
                            st["op"][:],
                            wo_sb[:, 1, db * P : (db + 1) * P],
                            u2q[:, 1, :],
                            start=True,
                            stop=True,
                        )
                    def cpb(db=db):
                        if db % 2 == 0:
                            st["osb"] = work.tile(
                                [P, 2, QT], F16, name="osb2", bufs=4
                            )
                        nc.vector.tensor_add(
                            st["osb"][:, db % 2, :], st["op"][:], parts[db][:]
                        )
                    us += [mmb, cpb]
                    if db % 2 == 1:
                        def dmb(db=db, qt=qt):
                            row = (qt * DC + db - 1) * P
                            dst = out_d[row : row + 2 * P, :].rearrange(
                                "(a p) q -> p a q", p=P
                            )
                            nc.sync.dma_start(dst, st["osb"][:])
                        us.append(dmb)
                return us

            units = []

            def fill(rounds_left):
                if not units:
                    return
                n = max(1, (len(units) + rounds_left - 1) // max(rounds_left, 1))
                for _ in range(min(n, len(units))):
                    units.pop(0)()

            def flush():
                while units:
                    units.pop(0)()

            def norm_store(qt, pr, accs, last=False):
                if not units:
                    # no filler left: pad the norm window with warm matmuls so
                    # PE_HAM doesn't see a low-duty window and re-throttle
                    for _ in range(2):
                        warm_mm()
                """Normalize the head pair and stack into u2[qt][:, pr, :]:
                even head straight from PSUM at lanes 0-63, odd head via a
                small SBUF->SBUF DMA hop to lanes 64-127 (scalar HWDGE ring;
                the gpsimd SWDGE queue's drain would gate the tail).

                For the last pair the whole chain is the kernel tail, so the
                odd head goes FIRST (its hop overlaps the even head's DVE
                mul) and one den evacuation moves to the idle ACT engine."""
                u2q = u2[qt]
                dens = {}
                for h2 in (1, 0) if last else (0, 1):
                    # f16 denominator stays on lane 64 (DVE can't move
                    # across partitions); the matmul streams from there
                    den = work.tile([DH + 1, QT], F16, name="den", bufs=6)
                    if last and h2 == 0:
                        nc.scalar.copy(den[DH : DH + 1, :], accs[h2][DH : DH + 1, :])
                    else:
                        nc.vector.tensor_copy(
                            den[DH : DH + 1, :], accs[h2][DH : DH + 1, :]
                        )
                    dens[h2] = den
                for h2 in (1, 0) if last else (0, 1):
                    rb_ps = _pj_tile([DH, QT], "rb")
                    nc.tensor.matmul(
                        rb_ps[:],
                        ones_f16[DH : DH + 1, :],
                        dens[h2][DH : DH + 1, :],
                        start=True,
                        stop=True,
                    )
                    rb_sb = work.tile([DH, QT], f32, name="rb_sb", bufs=4)
                    nc.vector.reciprocal_approx_fast(rb_sb[:], rb_ps[:])
                    if not units:
                        warm_mm()
                    if h2 == 0:
                        nc.vector.tensor_mul(
                            u2q[0:DH, pr, :], accs[h2][0:DH, :], rb_sb[:]
                        )
                    else:
                        tmp = work.tile([DH, QT], F16, name="u_tmp", bufs=2)
                        nc.vector.tensor_mul(tmp[:], accs[h2][0:DH, :], rb_sb[:])
                        nc.scalar.dma_start(u2q[DH:P, pr, :], tmp[:])

            # ---------------- main loop ----------------
            # upfront: only what gates qt0 pr0's first scores + first AV; the
            # remaining v chains and pr1's q/k chains become the first fillers
            for _ in range(7):
                warm_mm()
            for u in proj_units(0, prs=(0,), with_v=False) + v_chain_units(0):
                u()
            for qt in range(NQT):
                u2[qt] = work.tile([P, 2, QT], F16, name="u2", bufs=2)
                # filler work for this quarter's ACT-bound attention span
                if qt == 0:
                    for pc in range(1, 4):
                        units.extend(v_chain_units(pc))
                    units.extend(proj_units(0, prs=(1,), with_v=False))
                if qt + 1 < NQT:
                    units.extend(proj_units(qt + 1))
                if qt >= 1:
                    op_us = out_proj_units(qt - 1)
                    if qt == NQT - 1:
                        # hold back half of out_proj(2): it is the only
                        # dep-free PE work left to cover the last norm + DMA
                        # hop window after the final AV round
                        held_back = op_us[len(op_us) // 2 :]
                        op_us = op_us[: len(op_us) // 2]
                    units.extend(op_us)

                nk = (qt + 1) * (QT // P)
                # +3 slack keeps a few filler units in reserve past the last
                # round so the PE stays busy across the quarter-end norm
                # chain (a low-duty MID window would re-throttle HAM to 1.2
                # GHz for 3.4us)
                rounds_left = 2 * nk + 3
                for pr in range(2):
                    # full-partition shape so the tail out-projection can
                    # rotate its op tiles through these banks too
                    acc = [
                        ps_pool.tile(
                            [P, QT], f32, name=f"acc{h2}", tag=f"acc{h2}", bufs=1
                        )
                        for h2 in range(2)
                    ]

                    def av_round(kb, r, pat):
                        for h2 in range(2):
                            nc.tensor.matmul(
                                acc[h2][0 : DH + 1, r:QT],
                                v_aug[pr * 2 + h2][:, kb, :],
                                pat[:, h2 * QT + r : (h2 + 1) * QT],
                                start=(kb == 0),
                                stop=(kb == nk - 1),
                            )

                    pend = []  # [(kb, r, pat), ...] awaiting AV matmuls
                    for kb in range(nk):
                        k_sl = slice(kb * P, (kb + 1) * P)
                        ri = kb - qt * (QT // P)  # >= 0 on diagonal tiles
                        r = max(ri, 0) * P        # first valid col in this q tile
                        c_sl = slice(qt * QT + r, (qt + 1) * QT)
                        sc = ps_pool.tile([P, 2 * QT], f32, name="sc", tag="sc", bufs=2)
                        pat = work.tile([P, 2 * QT], F16, name="pat", bufs=4)
                        for h2 in range(2):
                            hb = h2 * DH
                            # explicit row-group placement: the two 64-row
                            # stationaries occupy disjoint halves of the PE
                            # array so their LDWEIGHTS+MATMULs can overlap
                            nc.tensor.matmul(
                                sc[:, h2 * QT + r : (h2 + 1) * QT],
                                kT[pr][hb : hb + DH, k_sl],
                                qT[pr][hb : hb + DH, c_sl],
                                start=True,
                                stop=True,
                                tile_position=(hb, 0),
                            )
                        # one exp covers both heads when the dead span (cols
                        # 512..512+r hold stale-PSUM garbage, never read) is
                        # small; on deep-diagonal rounds two exps are cheaper
                        # than exp'ing r columns of garbage
                        if r >= 2 * P:
                            for h2 in range(2):
                                e_sl = slice(h2 * QT + r, (h2 + 1) * QT)
                                nc.scalar.activation(
                                    pat[:, e_sl], sc[:, e_sl], Exp, scale=SCALE
                                )
                        else:
                            nc.scalar.activation(
                                pat[:, r : 2 * QT], sc[:, r : 2 * QT], Exp, scale=SCALE
                            )
                        if ri >= 0:
                            for h2 in range(2):
                                nc.vector.tensor_mul(
                                    pat[:, h2 * QT + r : h2 * QT + r + P],
                                    pat[:, h2 * QT + r : h2 * QT + r + P],
                                    tri_sb[:],
                                )
                        # software-pipeline the PE stream two rounds deep so
                        # the exp's ACT latency never exposes on the PE
                        pend.append((kb, r, pat))
                        if len(pend) > 3:
                            av_round(*pend.pop(0))
                        rounds_left -= 1
                        fill(rounds_left)
                    while pend:
                        av_round(*pend.pop(0))
                    norm_store(qt, pr, acc, last=(qt == NQT - 1 and pr == 1))
                    if qt == NQT - 1 and pr == 0:
                        a_us, op3_parts = out_proj_a_units(NQT - 1)
                        units.extend(a_us)
            for u in held_back:
                u()
            flush()
            for u in out_proj_b_units(NQT - 1, op3_parts):
                u()

    nc.compile()
    return nc


def _get_nc():
    if "nc" not in _CACHE:
        _CACHE["nc"] = _build()
    return _CACHE["nc"]


def _tri():
    k = np.arange(P)[:, None]
    q = np.arange(P)[None, :]
    return (q >= k).astype(np.float32)


def _ensure_ntff_hook():
    """Register the axon NTFF profile hook (missing antenv.axon_hooks shim)."""
    import types

    try:
        from antenv.axon_hooks import get_axon_ntff_profile_hook  # noqa: F401

        return
    except ImportError:
        pass
    import antenv

    if "/root/.axon_site" not in sys.path:
        sys.path.insert(0, "/root/.axon_site")
    from trn_agent_boot.trn_boot import _ntff_profile_via_ctypes

    hook = _ntff_profile_via_ctypes("/opt/axon/libaxon_pjrt.so")
    mod = types.ModuleType("antenv.axon_hooks")
    mod.get_axon_ntff_profile_hook = lambda: hook
    mod.set_axon_ntff_profile_hook = lambda h: None
    sys.modules["antenv.axon_hooks"] = mod
    antenv.axon_hooks = mod


def kernel(residual, W_Q, W_K, W_V, W_O):
    from concourse.bass_utils import run_bass_kernel_spmd

    if int(os.environ.get("KERNEL_TRACE", "0")):
        _ensure_ntff_hook()

    residual = np.ascontiguousarray(np.asarray(residual), np.float32)
    W_Q = np.ascontiguousarray(np.asarray(W_Q), np.float32)
    W_K = np.ascontiguousarray(np.asarray(W_K), np.float32)
    W_V = np.ascontiguousarray(np.asarray(W_V), np.float32)
    W_O = np.ascontiguousarray(np.asarray(W_O), np.float32)

    nc = _get_nc()
    tri = _tri()

    def chunked(a):
        # [D, n] -> [128, DC, n] so every DMA row is contiguous
        n = a.shape[1]
        return np.ascontiguousarray(
            a.reshape(DC, P, n).transpose(1, 0, 2).astype(np.float16)
        )

    in_maps = []
    for c in range(NCORES):
        b, g = divmod(c, GRP)
        hs = slice(g * NH_CORE, (g + 1) * NH_CORE)
        # pair-stack own-head W_O: [pair, odd, hd, D] -> [odd*64+hd, pair, D]
        wop = (
            W_O[hs]
            .reshape(2, 2, DH, D)
            .transpose(1, 2, 0, 3)
            .reshape(P, 2, D)
            .astype(np.float16)
        )
        in_maps.append(
            {
                "xt": chunked(residual[b].T),
                "wqt": chunked(W_Q[hs].transpose(2, 0, 1).reshape(D, NH_CORE * DH)),
                "wkt": chunked(W_K[hs].transpose(2, 0, 1).reshape(D, NH_CORE * DH)),
                "wvt": chunked(W_V[hs].transpose(2, 0, 1).reshape(D, NH_CORE * DH)),
                "wop": np.ascontiguousarray(wop),
                "msk": tri.astype(np.float16),
            }
        )

    res = run_bass_kernel_spmd(
        nc,
        in_maps,
        core_ids=list(range(NCORES)),
        trace=bool(int(os.environ.get("KERNEL_TRACE", "0"))),
        trace_cores=(
            list(range(NCORES))
            if int(os.environ.get("KERNEL_TRACE_ALL", "0"))
            else [0] if int(os.environ.get("KERNEL_TRACE", "0")) else None
        ),
    )
    _CACHE["last_results"] = res

    # host-side unshard: transpose each core's partial [qt, db, d, q] ->
    # [s, d] and sum the 4 head-group partials of each batch
    out = np.zeros((B, S, D), np.float32)
    for c in range(NCORES):
        b = c // GRP
        blk = np.asarray(res.results[c]["out"], np.float32).reshape(NQT, DC, P, QT)
        out[b] += blk.transpose(0, 3, 1, 2).reshape(S, D)
    return out


# revision 46
# speedup vs baseline: 1.0797x; 1.0137x over previous
"""Trainium2 Bass kernel for causal multi-head attention (B=2, S=2048, D=1024, 16 heads x 64).

Sharding: 8 cores = 2 batches x 4 head-groups (tensor parallel over heads),
collective-free. Each core computes attention for its 4 heads over the full
sequence AND applies its own 4-head slice of W_O to all q positions,
producing a transposed partial output [D, S] that the host transposes and
sums across the 4 cores of each batch. Moving the head-sum "all-reduce" to
the host removes all 8 on-device AllGathers (7-26us each) plus the CC
barrier and their tail serialization that dominated the previous design.

Attention is flash-style with transposed scores: sT[k, q] = K Q^T (keys on
partitions). The two heads of a pair run as tile-positioned 64-row
stationaries on disjoint PE halves (concurrent), writing one [128, 1024]
2-bank PSUM tile so a single ACT exp covers both (two exps on deep-diagonal
rounds where the dead middle span would exceed an instruction overhead). AV
uses stationary [v | 1] so PSUM row 64 accumulates the softmax denominator
for free; the denominator row is broadcast to 64 partitions by a matmul and
reciprocated at base partition 0 with the fast custom DVE op. The
normalized outputs of a head pair are stacked on partitions 0-63 / 64-127
of a u2 tile (the odd head hops through a small SBUF->SBUF DMA since DVE
cannot cross partitions), which makes the out-projection contraction a
clean pair of full-128-deep matmuls per 128-col output block.

Scheduling: QKV projection and out-projection chains are chopped into
single-instruction units and drained between attention rounds as PE filler
(with a few units held in reserve across quarter boundaries). The last
quarter's out-projection is split into pass A (pair 0, runs as filler
during the final attention pass) and pass B (pair 1 + add, PSUM rotated
through 4 banks so DVE evacuation never stalls the matmul stream); half of
out_proj(2) is held back to cover the final norm + DMA-hop window. Inputs
stream only on the two HWDGE rings (sync/scalar; gpsimd dma_start is the
slow SWDGE path), chunk-interleaved so the first projection chain starts as
soon as chunk 0 lands. Dummy full-array matmuls during the initial DMA wait
and dry norm windows keep PE_HAM's activity monitor busy so the PE clock
gate stays at 8/8 (2.4 GHz) instead of the 4/8 (1.2 GHz) cold state —
HAM tracks array duty, so quadrant-sized warm-ups do not register.
"""

import os
import sys

sys.path.insert(0, "/opt/trn_rl_repo")

import numpy as np

# ---- problem constants (hardcoded; kernel.py must be self-contained) ----
B = 2
S = 2048
D = 1024
N_HEADS = 16
DH = 64                 # head dim
NCORES = 8
NH_CORE = N_HEADS // 4  # 4 heads per core (4-way TP x 2-way batch DP)
SCALE = 1.0 / 8.0       # 1/sqrt(64)

P = 128                 # partitions
DC = D // P             # 8 contraction chunks for the projections
KC = S // P             # 16 key chunks
QT = 512                # q tile width (free dim) per quarter
NQT = S // QT           # 4 q tiles
GRP = 4                 # cores per batch group

_CACHE = {}


def _build():
    import concourse.bass as bass
    import concourse.tile as tile
    from concourse import bacc, mybir

    f32 = mybir.dt.float32
    F16 = mybir.dt.float16

    nc = bacc.Bacc(
        "TRN2",
        target_bir_lowering=False,
        debug=False,
        enable_asserts=False,
        num_devices=NCORES,
    )

    # all inputs pre-chunked host-side to [128, DC, n] so DMAs are contiguous
    xt_d = nc.dram_tensor("xt", [P, DC, S], F16, kind="ExternalInput").ap()
    wqt_d = nc.dram_tensor("wqt", [P, DC, NH_CORE * DH], F16, kind="ExternalInput").ap()
    wkt_d = nc.dram_tensor("wkt", [P, DC, NH_CORE * DH], F16, kind="ExternalInput").ap()
    wvt_d = nc.dram_tensor("wvt", [P, DC, NH_CORE * DH], F16, kind="ExternalInput").ap()
    # own-head W_O, pair-stacked: [128 partitions = (even head hd | odd head
    # hd), pair, D]
    wop_d = nc.dram_tensor("wop", [P, 2, D], F16, kind="ExternalInput").ap()
    msk_d = nc.dram_tensor("msk", [P, P], F16, kind="ExternalInput").ap()
    # transposed partial output: [qt, dblock, 128 d, 512 q]
    out_d = nc.dram_tensor("out", [NQT * DC * P, QT], F16, kind="ExternalOutput").ap()

    Exp = mybir.ActivationFunctionType.Exp

    with tile.TileContext(nc) as tc:
        with (
            tc.tile_pool(name="const", bufs=1) as const,
            tc.tile_pool(name="work", bufs=2) as work,
            tc.tile_pool(name="ps", bufs=1, space="PSUM") as ps_pool,
        ):
            # ---------------- input DMAs ----------------
            # Only the sync/scalar HWDGE rings move real bandwidth (gpsimd
            # dma_start is the slow SWDGE Q7 path); the two rings stream in
            # parallel and share the ~358 GB/s HBM port. Order by first use:
            # the q/k chains gate on (wq|wk) + xt quarter 0, the v chains on
            # wv, everything else trails.
            # wq and xt quarter 0 land per d-chunk, interleaved across the two
            # rings, so the first projection chain starts as soon as chunk 0
            # arrives and is then paced by chunk arrivals (not whole tensors)
            wq_sb = const.tile([P, DC, NH_CORE * DH], F16)
            xt_sb = const.tile([P, DC, S], F16)
            for dc in range(0, DC, 2):
                nc.sync.dma_start(wq_sb[:, dc : dc + 2, :], wqt_d[:, dc : dc + 2, :])
                nc.scalar.dma_start(
                    xt_sb[:, dc : dc + 2, 0:QT], xt_d[:, dc : dc + 2, 0:QT]
                )
            wk_sb = const.tile([P, DC, NH_CORE * DH], F16)
            nc.sync.dma_start(wk_sb[:, 0:4, :], wkt_d[:, 0:4, :])
            nc.sync.dma_start(wk_sb[:, 4:DC, :], wkt_d[:, 4:DC, :])
            wv_sb = const.tile([P, DC, NH_CORE * DH], F16)
            nc.sync.dma_start(wv_sb[:], wvt_d)
            tri_sb = const.tile([P, P], F16)
            nc.sync.dma_start(tri_sb[:], msk_d)
            half = (S - QT) // 2
            nc.scalar.dma_start(
                xt_sb[:, :, QT : QT + half], xt_d[:, :, QT : QT + half]
            )
            nc.sync.dma_start(
                xt_sb[:, :, QT + half : S], xt_d[:, :, QT + half : S]
            )
            wo_sb = const.tile([P, 2, D], F16)
            nc.scalar.dma_start(wo_sb[:], wop_d)

            # ---------------- SBUF state ----------------
            qT = [const.tile([P, S], F16, name=f"qT{i}") for i in range(2)]
            kT = [const.tile([P, S], F16, name=f"kT{i}") for i in range(2)]
            v_aug = [const.tile([P, KC, DH + 1], F16, name=f"vaug{h}") for h in range(NH_CORE)]
            ones_f32 = const.tile([P, DH], f32)
            nc.vector.memset(ones_f32[:], 1.0)
            ones_f16 = const.tile([DH + 1, DH], F16)
            nc.vector.memset(ones_f16[:], 1.0)
            for h in range(NH_CORE):
                nc.gpsimd.tensor_copy(v_aug[h][:, :, DH : DH + 1], ones_f32[:, 0:KC, None])

            # PE_HAM warm-up: dependency-free dummy matmuls keep the PE's
            # activity monitor busy through DMA stalls and norm chains so the
            # clock gate stays at 8/8 (2.4 GHz) instead of dropping to 4/8
            warm_mov = const.tile([P, QT], F16)
            nc.vector.memset(warm_mov[:], 0.0)
            warm_stat = const.tile([P, P], F16)
            nc.vector.memset(warm_stat[:], 0.0)
            warm_ct = [0]

            def warm_mm():
                # full 128x128 stationary: HAM appears to track PE array duty,
                # so a quadrant-sized warm matmul doesn't register as busy.
                # sc-tagged: that buffer is only ever held briefly (scores ->
                # exp), so a warm matmul can never deadlock against a
                # long-held projection accumulator the way a pj tile could
                warm_ct[0] += 1
                wp = ps_pool.tile([P, QT], f32, name="warm", tag="sc", bufs=2)
                nc.tensor.matmul(
                    wp[:],
                    warm_stat[:],
                    warm_mov[:],
                    start=True,
                    stop=True,
                )

            for _ in range(3):
                warm_mm()

            # per-quarter normalized pair-stacked attention outputs
            u2 = {}

            pj = [0]  # alternating tag counter for the 2 shared psum banks

            def _pj_tile(shape, name):
                t = ps_pool.tile(shape, f32, name=name, tag=f"pj{pj[0] % 2}", bufs=1)
                pj[0] += 1
                return t

            # ---- filler units: single instructions emitted between rounds ----
            def qk_chain_units(nt, pr, w_sb, dst):
                st = {}
                us = []
                for dc in range(DC):
                    def mm(dc=dc, nt=nt, pr=pr, w_sb=w_sb):
                        if dc == 0:
                            st["pp"] = _pj_tile([P, QT], "pp")
                        nc.tensor.matmul(
                            st["pp"][:],
                            w_sb[:, dc, pr * P : (pr + 1) * P],
                            xt_sb[:, dc, nt * QT : (nt + 1) * QT],
                            start=(dc == 0),
                            stop=(dc == DC - 1),
                        )
                    us.append(mm)
                def cp(nt=nt, dst=dst):
                    nc.vector.tensor_copy(dst[:, nt * QT : (nt + 1) * QT], st["pp"][:])
                us.append(cp)
                return us

            def v_chain_units(pc):
                st = {}
                us = []
                for dc in range(DC):
                    def mm(dc=dc, pc=pc):
                        if dc == 0:
                            st["vp"] = _pj_tile([P, NH_CORE * DH], "vp")
                        nc.tensor.matmul(
                            st["vp"][:],
                            xt_sb[:, dc, pc * P : (pc + 1) * P],
                            wv_sb[:, dc, :],
                            start=(dc == 0),
                            stop=(dc == DC - 1),
                        )
                    us.append(mm)
                for h in range(NH_CORE):
                    def cp(h=h, pc=pc):
                        nc.vector.tensor_copy(
                            v_aug[h][:, pc, 0:DH], st["vp"][:, h * DH : (h + 1) * DH]
                        )
                    us.append(cp)
                return us

            def proj_units(nt, prs=(0, 1), with_v=True):
                us = []
                for pr in prs:
                    us += qk_chain_units(nt, pr, wq_sb, qT[pr])
                    us += qk_chain_units(nt, pr, wk_sb, kT[pr])
                if with_v:
                    for pc in range(4 * nt, 4 * nt + 4):
                        us += v_chain_units(pc)
                return us

            def out_proj_units(qt):
                """Own-head out-projection for quarter qt: per d-block, two
                full-128-deep accumulating matmuls (one per head pair) over
                the pair-stacked u2(qt), then evacuate + store transposed."""
                u2q = u2[qt]
                st = {}
                us = []
                for db in range(DC):
                    def mm0(db=db):
                        st["op"] = _pj_tile([P, QT], "op")
                        nc.tensor.matmul(
                            st["op"][:],
                            wo_sb[:, 0, db * P : (db + 1) * P],
                            u2q[:, 0, :],
                            start=True,
                            stop=False,
                        )
                    def mm1(db=db):
                        nc.tensor.matmul(
                            st["op"][:],
                            wo_sb[:, 1, db * P : (db + 1) * P],
                            u2q[:, 1, :],
                            start=False,
                            stop=True,
                        )
                    def cp(db=db, qt=qt):
                        if db % 2 == 0:
                            st["osb"] = work.tile([P, 2, QT], F16, name="osb2", bufs=4)
                        nc.vector.tensor_copy(st["osb"][:, db % 2, :], st["op"][:])
                    us += [mm0, mm1, cp]
                    if db % 2 == 1:
                        def dm(db=db, qt=qt):
                            row = (qt * DC + db - 1) * P
                            dst = out_d[row : row + 2 * P, :].rearrange(
                                "(a p) q -> p a q", p=P
                            )
                            nc.sync.dma_start(dst, st["osb"][:])
                        us.append(dm)
                return us

            def out_proj_a_units(qt):
                """Tail-shortening pass A for the last quarter: project pair 0
                (complete after pr=0's norm) into f16 partials while pr=1's
                attention still runs."""
                u2q = u2[qt]
                st = {}
                parts = []
                us = []
                for db in range(DC):
                    def mma(db=db):
                        st["op"] = _pj_tile([P, QT], "op")
                        nc.tensor.matmul(
                            st["op"][:],
                            wo_sb[:, 0, db * P : (db + 1) * P],
                            u2q[:, 0, :],
                            start=True,
                            stop=True,
                        )
                    def cpa(db=db):
                        part = work.tile([P, QT], F16, name="opart", bufs=DC)
                        parts.append(part)
                        nc.vector.tensor_copy(part[:], st["op"][:])
                    us += [mma, cpa]
                return us, parts

            def out_proj_b_units(qt, parts):
                """Pass B: pair 1 matmul + add pass-A partial, evacuate, store.
                The op tiles rotate through 4 PSUM banks (pj pair + the now
                idle acc banks) so the matmul stream never stalls on a DVE
                evacuation; stores go out per 2 d-blocks."""
                u2q = u2[qt]
                st = {}
                tags = ["pj0", "pj1", "acc0", "acc1"]
                us = []
                for db in range(DC):
                    def mmb(db=db):
                        st["op"] = ps_pool.tile(
                            [P, QT], f32, name="opb", tag=tags[db % 4], bufs=1
                        )
                        nc.tensor.matmul(
                            st["op"][:],
                            wo_sb[:, 1, db * P : (db + 1) * P],
                            u2q[:, 1, :],
                            start=True,
                            stop=True,
                        )
                    def cpb(db=db):
                        if db % 2 == 0:
                            st["osb"] = work.tile(
                                [P, 2, QT], F16, name="osb2", bufs=4
                            )
                        nc.vector.tensor_add(
                            st["osb"][:, db % 2, :], st["op"][:], parts[db][:]
                        )
                    us += [mmb, cpb]
                    if db % 2 == 1:
                        def dmb(db=db, qt=qt):
                            row = (qt * DC + db - 1) * P
                            dst = out_d[row : row + 2 * P, :].rearrange(
                                "(a p) q -> p a q", p=P
                            )
                            nc.sync.dma_start(dst, st["osb"][:])
                        us.append(dmb)
                return us

            units = []

            def fill(rounds_left):
                if not units:
                    return
                n = max(1, (len(units) + rounds_left - 1) // max(rounds_left, 1))
                for _ in range(min(n, len(units))):
                    units.pop(0)()

            def flush():
                while units:
                    units.pop(0)()

            def norm_store(qt, pr, accs, last=False):
                if not units:
                    # no filler left: pad the norm window with warm matmuls so
                    # PE_HAM doesn't see a low-duty window and re-throttle
                    for _ in range(2):
                        warm_mm()
                """Normalize the head pair and stack into u2[qt][:, pr, :]:
                even head straight from PSUM at lanes 0-63, odd head via a
                small SBUF->SBUF DMA hop to lanes 64-127 (scalar HWDGE ring;
                the gpsimd SWDGE queue's drain would gate the tail).

                For the last pair the whole chain is the kernel tail, so the
                odd head goes FIRST (its hop overlaps the even head's DVE
                mul) and one den evacuation moves to the idle ACT engine."""
                u2q = u2[qt]
                dens = {}
                for h2 in (1, 0) if last else (0, 1):
                    # f16 denominator stays on lane 64 (DVE can't move
                    # across partitions); the matmul streams from there
                    den = work.tile([DH + 1, QT], F16, name="den", bufs=6)
                    if last and h2 == 0:
                        nc.scalar.copy(den[DH : DH + 1, :], accs[h2][DH : DH + 1, :])
                    else:
                        nc.vector.tensor_copy(
                            den[DH : DH + 1, :], accs[h2][DH : DH + 1, :]
                        )
                    dens[h2] = den
                for h2 in (1, 0) if last else (0, 1):
                    rb_ps = _pj_tile([DH, QT], "rb")
                    nc.tensor.matmul(
                        rb_ps[:],
                        ones_f16[DH : DH + 1, :],
                        dens[h2][DH : DH + 1, :],
                        start=True,
                        stop=True,
                    )
                    rb_sb = work.tile([DH, QT], f32, name="rb_sb", bufs=4)
                    nc.vector.reciprocal_approx_fast(rb_sb[:], rb_ps[:])
                    if not units:
                        warm_mm()
                    if h2 == 0:
                        nc.vector.tensor_mul(
                            u2q[0:DH, pr, :], accs[h2][0:DH, :], rb_sb[:]
                        )
                    else:
                        tmp = work.tile([DH, QT], F16, name="u_tmp", bufs=2)
                        nc.vector.tensor_mul(tmp[:], accs[h2][0:DH, :], rb_sb[:])
                        nc.scalar.dma_start(u2q[DH:P, pr, :], tmp[:])

            # ---------------- main loop ----------------
            # upfront: only what gates qt0 pr0's first scores + first AV; the
            # remaining v chains and pr1's q/k chains become the first fillers
            for _ in range(7):
                warm_mm()
            for u in proj_units(0, prs=(0,), with_v=False) + v_chain_units(0):
                u()
            for qt in range(NQT):
                u2[qt] = work.tile([P, 2, QT], F16, name="u2", bufs=2)
                # filler work for this quarter's ACT-bound attention span
                if qt == 0:
                    for pc in range(1, 4):
                        units.extend(v_chain_units(pc))
                    units.extend(proj_units(0, prs=(1,), with_v=False))
                if qt + 1 < NQT:
                    units.extend(proj_units(qt + 1))
                if qt >= 1:
                    op_us = out_proj_units(qt - 1)
                    if qt == NQT - 1:
                        # hold back half of out_proj(2): it is the only
                        # dep-free PE work left to cover the last norm + DMA
                        # hop window after the final AV round
                        held_back = op_us[len(op_us) // 2 :]
                        op_us = op_us[: len(op_us) // 2]
                    units.extend(op_us)

                nk = (qt + 1) * (QT // P)
                # +3 slack keeps a few filler units in reserve past the last
                # round so the PE stays busy across the quarter-end norm
                # chain (a low-duty MID window would re-throttle HAM to 1.2
                # GHz for 3.4us)
                rounds_left = 2 * nk + 3
                for pr in range(2):
                    # full-partition shape so the tail out-projection can
                    # rotate its op tiles through these banks too
                    acc = [
                        ps_pool.tile(
                            [P, QT], f32, name=f"acc{h2}", tag=f"acc{h2}", bufs=1
                        )
                        for h2 in range(2)
                    ]

                    def av_round(kb, r, pat):
                        for h2 in range(2):
                            nc.tensor.matmul(
                                acc[h2][0 : DH + 1, r:QT],
                                v_aug[pr * 2 + h2][:, kb, :],
                                pat[:, h2 * QT + r : (h2 + 1) * QT],
                                start=(kb == 0),
                                stop=(kb == nk - 1),
                            )

                    pend = []  # [(kb, r, pat), ...] awaiting AV matmuls
                    for kb in range(nk):
                        k_sl = slice(kb * P, (kb + 1) * P)
                        ri = kb - qt * (QT // P)  # >= 0 on diagonal tiles
                        r = max(ri, 0) * P        # first valid col in this q tile
                        c_sl = slice(qt * QT + r, (qt + 1) * QT)
                        sc = ps_pool.tile([P, 2 * QT], f32, name="sc", tag="sc", bufs=2)
                        pat = work.tile([P, 2 * QT], F16, name="pat", bufs=4)
                        for h2 in range(2):
                            hb = h2 * DH
                            # explicit row-group placement: the two 64-row
                            # stationaries occupy disjoint halves of the PE
                            # array so their LDWEIGHTS+MATMULs can overlap
                            nc.tensor.matmul(
                                sc[:, h2 * QT + r : (h2 + 1) * QT],
                                kT[pr][hb : hb + DH, k_sl],
                                qT[pr][hb : hb + DH, c_sl],
                                start=True,
                                stop=True,
                                tile_position=(hb, 0),
                            )
                        # one exp covers both heads when the dead span (cols
                        # 512..512+r hold stale-PSUM garbage, never read) is
                        # small; on deep-diagonal rounds two exps are cheaper
                        # than exp'ing r columns of garbage
                        if r >= 2 * P:
                            for h2 in range(2):
                                e_sl = slice(h2 * QT + r, (h2 + 1) * QT)
                                nc.scalar.activation(
                                    pat[:, e_sl], sc[:, e_sl], Exp, scale=SCALE
                                )
                        else:
                            nc.scalar.activation(
                                pat[:, r : 2 * QT], sc[:, r : 2 * QT], Exp, scale=SCALE
                            )
                        if ri >= 0:
                            for h2 in range(2):
                                nc.vector.tensor_mul(
                                    pat[:, h2 * QT + r : h2 * QT + r + P],
                                    pat[:, h2 * QT + r : h2 * QT + r + P],
                                    tri_sb[:],
                                )
                        # software-pipeline the PE stream two rounds deep so
                        # the exp's ACT latency never exposes on the PE
                        pend.append((kb, r, pat))
                        if len(pend) > 4:
                            av_round(*pend.pop(0))
                        rounds_left -= 1
                        fill(rounds_left)
                    while pend:
                        av_round(*pend.pop(0))
                    norm_store(qt, pr, acc, last=(qt == NQT - 1 and pr == 1))
                    if qt == NQT - 1 and pr == 0:
                        a_us, op3_parts = out_proj_a_units(NQT - 1)
                        units.extend(a_us)
            for u in held_back:
                u()
            flush()
            for u in out_proj_b_units(NQT - 1, op3_parts):
                u()

    nc.compile()
    return nc


def _get_nc():
    if "nc" not in _CACHE:
        _CACHE["nc"] = _build()
    return _CACHE["nc"]


def _tri():
    k = np.arange(P)[:, None]
    q = np.arange(P)[None, :]
    return (q >= k).astype(np.float32)


def _ensure_ntff_hook():
    """Register the axon NTFF profile hook (missing antenv.axon_hooks shim)."""
    import types

    try:
        from antenv.axon_hooks import get_axon_ntff_profile_hook  # noqa: F401

        return
    except ImportError:
        pass
    import antenv

    if "/root/.axon_site" not in sys.path:
        sys.path.insert(0, "/root/.axon_site")
    from trn_agent_boot.trn_boot import _ntff_profile_via_ctypes

    hook = _ntff_profile_via_ctypes("/opt/axon/libaxon_pjrt.so")
    mod = types.ModuleType("antenv.axon_hooks")
    mod.get_axon_ntff_profile_hook = lambda: hook
    mod.set_axon_ntff_profile_hook = lambda h: None
    sys.modules["antenv.axon_hooks"] = mod
    antenv.axon_hooks = mod


def kernel(residual, W_Q, W_K, W_V, W_O):
    from concourse.bass_utils import run_bass_kernel_spmd

    if int(os.environ.get("KERNEL_TRACE", "0")):
        _ensure_ntff_hook()

    residual = np.ascontiguousarray(np.asarray(residual), np.float32)
    W_Q = np.ascontiguousarray(np.asarray(W_Q), np.float32)
    W_K = np.ascontiguousarray(np.asarray(W_K), np.float32)
    W_V = np.ascontiguousarray(np.asarray(W_V), np.float32)
    W_O = np.ascontiguousarray(np.asarray(W_O), np.float32)

    nc = _get_nc()
    tri = _tri()

    def chunked(a):
        # [D, n] -> [128, DC, n] so every DMA row is contiguous
        n = a.shape[1]
        return np.ascontiguousarray(
            a.reshape(DC, P, n).transpose(1, 0, 2).astype(np.float16)
        )

    in_maps = []
    for c in range(NCORES):
        b, g = divmod(c, GRP)
        hs = slice(g * NH_CORE, (g + 1) * NH_CORE)
        # pair-stack own-head W_O: [pair, odd, hd, D] -> [odd*64+hd, pair, D]
        wop = (
            W_O[hs]
            .reshape(2, 2, DH, D)
            .transpose(1, 2, 0, 3)
            .reshape(P, 2, D)
            .astype(np.float16)
        )
        in_maps.append(
            {
                "xt": chunked(residual[b].T),
                "wqt": chunked(W_Q[hs].transpose(2, 0, 1).reshape(D, NH_CORE * DH)),
                "wkt": chunked(W_K[hs].transpose(2, 0, 1).reshape(D, NH_CORE * DH)),
                "wvt": chunked(W_V[hs].transpose(2, 0, 1).reshape(D, NH_CORE * DH)),
                "wop": np.ascontiguousarray(wop),
                "msk": tri.astype(np.float16),
            }
        )

    res = run_bass_kernel_spmd(
        nc,
        in_maps,
        core_ids=list(range(NCORES)),
        trace=bool(int(os.environ.get("KERNEL_TRACE", "0"))),
        trace_cores=(
            list(range(NCORES))
            if int(os.environ.get("KERNEL_TRACE_ALL", "0"))
            else [0] if int(os.environ.get("KERNEL_TRACE", "0")) else None
        ),
    )
    _CACHE["last_results"] = res

    # host-side unshard: transpose each core's partial [qt, db, d, q] ->
    # [s, d] and sum the 4 head-group partials of each batch
    out = np.zeros((B, S, D), np.float32)
    for c in range(NCORES):
        b = c // GRP
        blk = np.asarray(res.results[c]["out"], np.float32).reshape(NQT, DC, P, QT)
        out[b] += blk.transpose(0, 3, 1, 2).reshape(S, D)
    return out
